# revision 16
# baseline (speedup 1.0000x reference)
"""Trainium2 Bass kernel for grouped-query attention with qk-norm.

Problem (hardcoded): x(2,2048,1024) @ Wq(1024,1024) / Wkv(1024,512),
16 query heads, 4 kv heads, head_dim 64, k_scale(16,1,64) applied to the
group-broadcast k. Output (2,2048,1024).

Sharding: 8 cores = batch(2) x kv_heads(4). Each core computes its batch's
4 query heads against its kv head over the full 2048x2048 score matrix.

Device kernel layout choices:
- Host passes x transposed (xT: dim on partitions) so all projection
  matmuls contract over dim with no on-device transposes.
- k_scale is folded into Wk host-side: (x@Wk)*ks == x@(Wk*diag(ks)),
  giving a per-query-head scaled kkT directly from the projection.
- Scores are computed transposed (S^T: keys on partitions, queries free)
  so that exp(S^T) tiles feed the PV matmul directly as the moving
  operand (no P transpose).
- Softmax skips the max-subtraction (inputs are bounded; exp stays well
  inside fp32 range) and normalizes after PV via an appended ones-row in
  the V stationary operand (row 64 of the PV psum accumulates sum(exp)).
- Output is returned transposed per head (oT: 4*64 x 2048); the host
  transposes during the gather.
- Matmul inputs are float32r (fp32 storage, reduced-precision multiply,
  4x the fp32 throughput at moving-dim >= 256).
"""

from contextlib import ExitStack

import numpy as np

import concourse.bacc as bacc
import concourse.mybir as mybir
import concourse.tile as tile
from concourse.bass_utils import run_bass_kernel_spmd

# Problem constants
B, N, DIM = 2, 2048, 1024
HEADS, KV_HEADS, DH = 16, 4, 64
G = HEADS // KV_HEADS  # query heads per kv head (4)
NCORES = 8
P = 128
KT = DIM // P  # 8 contraction tiles over dim
IC = 512  # query-chunk width
NI = N // IC  # 4
NJ = N // P  # 16 key tiles
SCALE = DH**-0.5

F32 = mybir.dt.float32
F32R = mybir.dt.float32r
F16 = mybir.dt.float16

# matmul input dtype: fp16 streams 1 row/cycle (fp32r only manages 1 row per
# 2 cycles — measured ~500ns vs ~215ns for a 512-wide moving operand)
DMM = F16


def emit_kernel(ctx, tc, xT, wq, wk, wv, eye, oT):
    nc = tc.nc
    Exp = mybir.ActivationFunctionType.Exp
    mult = mybir.AluOpType.mult

    wpool = ctx.enter_context(tc.tile_pool(name="w", bufs=1))
    qkpool = ctx.enter_context(tc.tile_pool(name="qk", bufs=1))
    ptpool = ctx.enter_context(tc.tile_pool(name="pt", bufs=6))
    npool = ctx.enter_context(tc.tile_pool(name="norm", bufs=2))

    # --- persistent SBUF tensors ---
    ones_sb = wpool.tile([P, DH], DMM, tag="ones")  # 1-row slices as bc lhsT
    eye_sb = wpool.tile([DH, DH], DMM, tag="eye")  # identity for vT transpose
    qT = [qkpool.tile([P, N], DMM, name=f"qT{hp}", tag=f"qT{hp}") for hp in range(2)]
    kkT = [qkpool.tile([P, N], DMM, name=f"kkT{hp}", tag=f"kkT{hp}") for hp in range(2)]
    vaug = qkpool.tile([P, NJ * (DH + 1)], F16, tag="vaug")
    nc.any.memset(vaug[:], 1.0)
    nc.any.memset(ones_sb[:], 1.0)
    warm = qkpool.tile([1, 2], F32, tag="warm")
    nc.any.memset(warm[0:1, 0:1], 0.0)
    nc.scalar.activation(warm[0:1, 1:2], warm[0:1, 0:1], Exp)
    nc.sync.dma_start(eye_sb[:], eye[:, :])

    def normalize_half(h, ic, o_psum):
        # sums row -> SBUF, GpSimd broadcast across partitions (PE-free),
        # reciprocal on the broadcast (cheap at 64 partitions), multiply the
        # PV accumulator in place in PSUM, DMA out. No DRAM bounce.
        csl = slice(ic * IC, (ic + 1) * IC)
        srow = npool.tile([1, IC], F32, tag="srow", bufs=4)
        nc.vector.tensor_copy(srow[:], o_psum[DH : DH + 1, :])
        bc = npool.tile([DH, IC], F32, name="bcg", tag="bcg", bufs=4)
        nc.gpsimd.partition_broadcast(bc[:], srow[:])
        rb = npool.tile([DH, IC], F32, tag="rb", bufs=4)
        nc.vector.reciprocal(rb[:], bc[:])
        fin = npool.tile([DH, IC], F32, tag="fin", bufs=4)
        nc.vector.tensor_tensor(fin[:], o_psum[0:DH, :], rb[:], mult)
        nc.sync.dma_start(oT[h * DH : (h + 1) * DH, csl], fin[:])

    def qk_exp(hp, ic, jt, pt):
        csl = slice(ic * IC, (ic + 1) * IC)
        st = apsum.tile([P, 2 * IC], F32, tag="s", bufs=2, name="st")
        for half in range(2):
            rsl = slice(half * 64, half * 64 + 64)
            nc.tensor.matmul(
                st[:, half * IC : (half + 1) * IC],
                kkT[hp][rsl, jt * P : (jt + 1) * P],
                qT[hp][rsl, csl],
                start=True,
                stop=True,
                tile_position=(half * 64, 0),
            )
        nc.scalar.activation(pt[:], st[:], Exp, scale=SCALE)

    def pv_mm(o_ps, jt, pt):
        for half in range(2):
            nc.tensor.matmul(
                o_ps[half][:],
                vaug[:, jt * (DH + 1) : (jt + 1) * (DH + 1)],
                pt[:, half * IC : (half + 1) * IC],
                start=(jt == 0),
                stop=(jt == NJ - 1),
            )

    def attn_block(hp, ic, o_ps, jts):
        for jt in jts:
            pt = ptpool.tile([P, 2 * IC], F16, tag="pt")
            qk_exp(hp, ic, jt, pt)
            pv_mm(o_ps, jt, pt)

    def drain_block(hp, ic, o_ps):
        for half in range(2):
            normalize_half(2 * hp + half, ic, o_ps[half])

    # S-tile pool lives for the whole kernel so early attention blocks can
    # overlap the projection phase (PV is deferred; its accumulator banks
    # open only after the projection psum pool closes).
    apsum = ctx.enter_context(tc.tile_pool(name="ap", bufs=2, space="PSUM"))
    # Dummy matmuls during the initial DMA wait keep the PE HAM activity
    # monitor busy so real projections start at 2.4GHz instead of 1.2.
    for _ in range(28):
        wt = apsum.tile([DH, IC], F32, tag="s", name="wt", bufs=2)
        nc.tensor.matmul(
            wt[:, 0:DH], ones_sb[:, 0:DH], ones_sb[:, 0:DH],
            start=True, stop=True,
        )

    # --- projections (fp16 inputs): qT / kkT (d on partitions) + vT ---
    with tc.tile_pool(name="xw", bufs=1) as xwpool:
        wq_sb = xwpool.tile([P, KT * 256], F16, tag="wq")
        wk_sb = xwpool.tile([P, KT * 256], F16, tag="wk")
        wv_sb = xwpool.tile([P, KT * DH], F16, tag="wv")
        xts = xwpool.tile([P, KT * N], F16, tag="xt")  # 4MB
        vT_sb = xwpool.tile([DH, N], DMM, tag="vT")

        def dma_x(kt, ic):
            r = slice(kt * P, (kt + 1) * P)
            csl = slice(ic * IC, (ic + 1) * IC)
            nc.gpsimd.dma_start(
                xts[:, kt * N + ic * IC : kt * N + (ic + 1) * IC], xT[r, csl]
            )

        for kt in range(KT):
            r = slice(kt * P, (kt + 1) * P)
            nc.sync.dma_start(wq_sb[:, kt * 256 : (kt + 1) * 256], wq[r, :])
            dma_x(kt, 0)
        for kt in range(KT):
            r = slice(kt * P, (kt + 1) * P)
            nc.sync.dma_start(wk_sb[:, kt * 256 : (kt + 1) * 256], wk[r, :])
            dma_x(kt, 1)
        for kt in range(KT):
            r = slice(kt * P, (kt + 1) * P)
            nc.sync.dma_start(wv_sb[:, kt * DH : (kt + 1) * DH], wv[r, :])
            dma_x(kt, 2)
        for kt in range(KT):
            dma_x(kt, 3)

        def proj_wave(ic, pp):
            # one wave = every projection chain that consumes xts chunk ic
            csl = slice(ic * IC, (ic + 1) * IC)
            for hp in range(2):
                for t, w_sb in ((qT[hp], wq_sb), (kkT[hp], wk_sb)):
                    ps = pp.tile([P, IC], F32, tag="pj", name="pjt", bufs=2)
                    for kt in range(KT):
                        c0 = kt * 256 + hp * 128
                        nc.tensor.matmul(
                            ps[:],
                            w_sb[:, c0 : c0 + 128],
                            xts[:, kt * N + ic * IC : kt * N + (ic + 1) * IC],
                            start=(kt == 0),
                            stop=(kt == KT - 1),
                        )
                    nc.vector.tensor_copy(t[:, csl], ps[:])
            ps = pp.tile([DH, IC], F32, tag="pj", name="pjv", bufs=2)
            for kt in range(KT):
                nc.tensor.matmul(
                    ps[:],
                    wv_sb[:, kt * DH : (kt + 1) * DH],
                    xts[:, kt * N + ic * IC : kt * N + (ic + 1) * IC],
                    start=(kt == 0),
                    stop=(kt == KT - 1),
                )
            nc.vector.tensor_copy(vT_sb[:, csl], ps[:])
            for jt in range(4 * ic, 4 * ic + 4):
                pv = pp.tile([P, DH], DMM, tag="pj", bufs=2, name="pvt")
                nc.tensor.transpose(
                    pv[:], vT_sb[:, jt * P : (jt + 1) * P], eye_sb[:]
                )
                nc.vector.tensor_copy(
                    vaug[:, jt * (DH + 1) : jt * (DH + 1) + DH], pv[:]
                )

        pt_hold = [
            ptpool.tile([P, 2 * IC], F16, name=f"pth{j}", tag=f"pth{j}", bufs=1)
            for j in range(8)
        ]
        with tc.tile_pool(name="pp", bufs=2, space="PSUM") as pp:
            proj_wave(0, pp)
            proj_wave(1, pp)
            # early QK+exp for (hp0, ic0) j-tiles 0-7 overlap the remaining
            # projection waves; their PV runs later (accumulator banks are
            # still occupied by the projection pool here).
            for jt in range(8):
                qk_exp(0, 0, jt, pt_hold[jt])
            proj_wave(2, pp)
            proj_wave(3, pp)

    # --- attention ---
    with tc.tile_pool(name="op", bufs=2, space="PSUM") as opool:
        for hp in range(2):
            for ic in range(NI):
                o_ps = [
                    opool.tile(
                        [DH + 1, IC], F32, name=f"ops{i}", tag=f"ops{i}", bufs=2
                    )
                    for i in range(2)
                ]
                if hp == 0 and ic == 0:
                    for jt in range(8):
                        pv_mm(o_ps, jt, pt_hold[jt])
                    attn_block(hp, ic, o_ps, range(8, NJ))
                else:
                    attn_block(hp, ic, o_ps, range(NJ))
                drain_block(hp, ic, o_ps)


_CACHE = {}


def build():
    if "nc" in _CACHE:
        return _CACHE["nc"]
    nc = bacc.Bacc(
        "TRN2", target_bir_lowering=False, debug=False, num_devices=NCORES
    )
    xT = nc.dram_tensor("xT", (DIM, N), F16, kind="ExternalInput").ap()
    wq = nc.dram_tensor("wq", (DIM, G * DH), F16, kind="ExternalInput").ap()
    wk = nc.dram_tensor("wk", (DIM, G * DH), F16, kind="ExternalInput").ap()
    wv = nc.dram_tensor("wv", (DIM, DH), F16, kind="ExternalInput").ap()
    eye = nc.dram_tensor("eye", (DH, DH), F16, kind="ExternalInput").ap()
    oT = nc.dram_tensor("oT", (G * DH, N), F32, kind="ExternalOutput").ap()
    with tile.TileContext(nc) as tc:
        with ExitStack() as ctx:
            emit_kernel(ctx, tc, xT, wq, wk, wv, eye, oT)
    nc.compile()
    _CACHE["nc"] = nc
    return nc


def make_in_maps(x, Wq, Wkv, k_scale):
    x = np.asarray(x, dtype=np.float32)
    Wq = np.asarray(Wq, dtype=np.float32)
    Wkv = np.asarray(Wkv, dtype=np.float32)
    k_scale = np.asarray(k_scale, dtype=np.float32)
    xTs = [np.ascontiguousarray(x[b].T) for b in range(B)]
    in_maps = []
    for c in range(NCORES):
        b, kv = divmod(c, KV_HEADS)
        wk_base = Wkv[:, kv * DH : (kv + 1) * DH]
        wk_c = np.concatenate(
            [wk_base * k_scale[kv * G + j, 0][None, :] for j in range(G)], axis=1
        )
        in_maps.append(
            {
                "xT": xTs[b].astype(np.float16),
                "wq": np.ascontiguousarray(Wq[:, kv * G * DH : (kv + 1) * G * DH]).astype(np.float16),
                "wk": np.ascontiguousarray(wk_c).astype(np.float16),
                "wv": np.ascontiguousarray(
                    Wkv[:, KV_HEADS * DH + kv * DH : KV_HEADS * DH + (kv + 1) * DH]
                ).astype(np.float16),
                "eye": np.eye(DH, dtype=np.float16),
            }
        )
    return in_maps


def gather(results):
    out = np.empty((B, N, HEADS * DH), dtype=np.float32)
    for c in range(NCORES):
        b, kv = divmod(c, KV_HEADS)
        out[b, :, kv * G * DH : (kv + 1) * G * DH] = results[c]["oT"].T
    return out


def kernel(x, Wq, Wkv, k_scale, _trace=False):
    nc = build()
    in_maps = make_in_maps(x, Wq, Wkv, k_scale)
    res = run_bass_kernel_spmd(
        nc, in_maps, core_ids=list(range(NCORES)), trace=_trace
    )
    out = gather(res.results)
    if _trace:
        kernel.last_result = res
    return out



# revision 18
# speedup vs baseline: 1.0212x; 1.0212x over previous
"""Trainium2 Bass kernel for grouped-query attention with qk-norm.

Problem (hardcoded): x(2,2048,1024) @ Wq(1024,1024) / Wkv(1024,512),
16 query heads, 4 kv heads, head_dim 64, k_scale(16,1,64) applied to the
group-broadcast k. Output (2,2048,1024).

Sharding: 8 cores = batch(2) x kv_heads(4). Each core computes its batch's
4 query heads against its kv head over the full 2048x2048 score matrix.

Device kernel layout choices:
- Host passes x transposed (xT: dim on partitions) so all projection
  matmuls contract over dim with no on-device transposes.
- k_scale is folded into Wk host-side: (x@Wk)*ks == x@(Wk*diag(ks)),
  giving a per-query-head scaled kkT directly from the projection.
- Scores are computed transposed (S^T: keys on partitions, queries free)
  so that exp(S^T) tiles feed the PV matmul directly as the moving
  operand (no P transpose).
- Softmax skips the max-subtraction (inputs are bounded; exp stays well
  inside fp32 range) and normalizes after PV via an appended ones-row in
  the V stationary operand (row 64 of the PV psum accumulates sum(exp)).
- Output is returned transposed per head (oT: 4*64 x 2048); the host
  transposes during the gather.
- Matmul inputs are float32r (fp32 storage, reduced-precision multiply,
  4x the fp32 throughput at moving-dim >= 256).
"""

from contextlib import ExitStack

import numpy as np

import concourse.bacc as bacc
import concourse.mybir as mybir
import concourse.tile as tile
from concourse.bass_utils import run_bass_kernel_spmd

# Problem constants
B, N, DIM = 2, 2048, 1024
HEADS, KV_HEADS, DH = 16, 4, 64
G = HEADS // KV_HEADS  # query heads per kv head (4)
NCORES = 8
P = 128
KT = DIM // P  # 8 contraction tiles over dim
IC = 512  # query-chunk width
NI = N // IC  # 4
NJ = N // P  # 16 key tiles
SCALE = DH**-0.5

F32 = mybir.dt.float32
F32R = mybir.dt.float32r
F16 = mybir.dt.float16

# matmul input dtype: fp16 streams 1 row/cycle (fp32r only manages 1 row per
# 2 cycles — measured ~500ns vs ~215ns for a 512-wide moving operand)
DMM = F16


def emit_kernel(ctx, tc, xT, wq, wk, wv, eye, oT):
    nc = tc.nc
    Exp = mybir.ActivationFunctionType.Exp
    mult = mybir.AluOpType.mult

    wpool = ctx.enter_context(tc.tile_pool(name="w", bufs=1))
    qkpool = ctx.enter_context(tc.tile_pool(name="qk", bufs=1))
    ptpool = ctx.enter_context(tc.tile_pool(name="pt", bufs=6))
    npool = ctx.enter_context(tc.tile_pool(name="norm", bufs=2))

    # --- persistent SBUF tensors ---
    ones_sb = wpool.tile([P, DH], DMM, tag="ones")  # 1-row slices as bc lhsT
    eye_sb = wpool.tile([DH, DH], DMM, tag="eye")  # identity for vT transpose
    qT = [qkpool.tile([P, N], DMM, name=f"qT{hp}", tag=f"qT{hp}") for hp in range(2)]
    kkT = [qkpool.tile([P, N], DMM, name=f"kkT{hp}", tag=f"kkT{hp}") for hp in range(2)]
    vaug = qkpool.tile([P, NJ * (DH + 1)], F16, tag="vaug")
    nc.any.memset(vaug[:], 1.0)
    nc.any.memset(ones_sb[:], 1.0)
    warm = qkpool.tile([1, 2], F32, tag="warm")
    nc.any.memset(warm[0:1, 0:1], 0.0)
    nc.scalar.activation(warm[0:1, 1:2], warm[0:1, 0:1], Exp)
    nc.sync.dma_start(eye_sb[:], eye[:, :])

    def normalize_half(h, ic, o_psum):
        # sums row -> SBUF, GpSimd broadcast across partitions (PE-free),
        # single-pass approx reciprocal (~18 bits) on the broadcast,
        # multiply the PV accumulator, DMA out. No DRAM bounce.
        csl = slice(ic * IC, (ic + 1) * IC)
        srow = npool.tile([1, IC], F32, tag="srow", bufs=4)
        nc.vector.tensor_copy(srow[:], o_psum[DH : DH + 1, :])
        bc = npool.tile([DH, IC], F32, name="bcg", tag="bcg", bufs=4)
        nc.gpsimd.partition_broadcast(bc[:], srow[:])
        rb = npool.tile([DH, IC], F32, tag="rb", bufs=4)
        nc.vector.reciprocal_approx_fast(rb[:], bc[:])
        fin = npool.tile([DH, IC], F32, tag="fin", bufs=4)
        nc.vector.tensor_tensor(fin[:], o_psum[0:DH, :], rb[:], mult)
        nc.sync.dma_start(oT[h * DH : (h + 1) * DH, csl], fin[:])

    def qk_exp(hp, ic, jt, pt):
        csl = slice(ic * IC, (ic + 1) * IC)
        st = apsum.tile([P, 2 * IC], F32, tag="s", bufs=2, name="st")
        for half in range(2):
            rsl = slice(half * 64, half * 64 + 64)
            nc.tensor.matmul(
                st[:, half * IC : (half + 1) * IC],
                kkT[hp][rsl, jt * P : (jt + 1) * P],
                qT[hp][rsl, csl],
                start=True,
                stop=True,
                tile_position=(half * 64, 0),
            )
        nc.scalar.activation(pt[:], st[:], Exp, scale=SCALE)

    def pv_mm(o_ps, jt, pt):
        for half in range(2):
            nc.tensor.matmul(
                o_ps[half][:],
                vaug[:, jt * (DH + 1) : (jt + 1) * (DH + 1)],
                pt[:, half * IC : (half + 1) * IC],
                start=(jt == 0),
                stop=(jt == NJ - 1),
            )

    def attn_block(hp, ic, o_ps, jts):
        for jt in jts:
            pt = ptpool.tile([P, 2 * IC], F16, tag="pt")
            qk_exp(hp, ic, jt, pt)
            pv_mm(o_ps, jt, pt)

    def drain_block(hp, ic, o_ps):
        for half in range(2):
            normalize_half(2 * hp + half, ic, o_ps[half])

    # S-tile pool lives for the whole kernel so early attention blocks can
    # overlap the projection phase (PV is deferred; its accumulator banks
    # open only after the projection psum pool closes).
    apsum = ctx.enter_context(tc.tile_pool(name="ap", bufs=2, space="PSUM"))
    # Dummy matmuls during the initial DMA wait keep the PE HAM activity
    # monitor busy so real projections start at 2.4GHz instead of 1.2.
    for _ in range(28):
        wt = apsum.tile([DH, IC], F32, tag="s", name="wt", bufs=2)
        nc.tensor.matmul(
            wt[:, 0:DH], ones_sb[:, 0:DH], ones_sb[:, 0:DH],
            start=True, stop=True,
        )

    # --- projections (fp16 inputs): qT / kkT (d on partitions) + vT ---
    with tc.tile_pool(name="xw", bufs=1) as xwpool:
        wq_sb = xwpool.tile([P, KT * 256], F16, tag="wq")
        wk_sb = xwpool.tile([P, KT * 256], F16, tag="wk")
        wv_sb = xwpool.tile([P, KT * DH], F16, tag="wv")
        xts = xwpool.tile([P, KT * N], F16, tag="xt")  # 4MB
        vT_sb = xwpool.tile([DH, N], DMM, tag="vT")

        def dma_x(kt, ic):
            r = slice(kt * P, (kt + 1) * P)
            csl = slice(ic * IC, (ic + 1) * IC)
            nc.gpsimd.dma_start(
                xts[:, kt * N + ic * IC : kt * N + (ic + 1) * IC], xT[r, csl]
            )

        for kt in range(KT):
            r = slice(kt * P, (kt + 1) * P)
            nc.sync.dma_start(wq_sb[:, kt * 256 : (kt + 1) * 256], wq[r, :])
            dma_x(kt, 0)
        for kt in range(KT):
            r = slice(kt * P, (kt + 1) * P)
            nc.sync.dma_start(wk_sb[:, kt * 256 : (kt + 1) * 256], wk[r, :])
            dma_x(kt, 1)
        for kt in range(KT):
            r = slice(kt * P, (kt + 1) * P)
            nc.sync.dma_start(wv_sb[:, kt * DH : (kt + 1) * DH], wv[r, :])
            dma_x(kt, 2)
        for kt in range(KT):
            dma_x(kt, 3)

        def proj_wave(ic, pp):
            # one wave = every projection chain that consumes xts chunk ic
            csl = slice(ic * IC, (ic + 1) * IC)
            for hp in range(2):
                for t, w_sb in ((qT[hp], wq_sb), (kkT[hp], wk_sb)):
                    ps = pp.tile([P, IC], F32, tag="pj", name="pjt", bufs=2)
                    for kt in range(KT):
                        c0 = kt * 256 + hp * 128
                        nc.tensor.matmul(
                            ps[:],
                            w_sb[:, c0 : c0 + 128],
                            xts[:, kt * N + ic * IC : kt * N + (ic + 1) * IC],
                            start=(kt == 0),
                            stop=(kt == KT - 1),
                        )
                    nc.vector.tensor_copy(t[:, csl], ps[:])
            ps = pp.tile([DH, IC], F32, tag="pj", name="pjv", bufs=2)
            for kt in range(KT):
                nc.tensor.matmul(
                    ps[:],
                    wv_sb[:, kt * DH : (kt + 1) * DH],
                    xts[:, kt * N + ic * IC : kt * N + (ic + 1) * IC],
                    start=(kt == 0),
                    stop=(kt == KT - 1),
                )
            nc.vector.tensor_copy(vT_sb[:, csl], ps[:])
            for jt in range(4 * ic, 4 * ic + 4):
                pv = pp.tile([P, DH], DMM, tag="pj", bufs=2, name="pvt")
                nc.tensor.transpose(
                    pv[:], vT_sb[:, jt * P : (jt + 1) * P], eye_sb[:]
                )
                nc.vector.tensor_copy(
                    vaug[:, jt * (DH + 1) : jt * (DH + 1) + DH], pv[:]
                )

        pt_hold = [
            ptpool.tile([P, 2 * IC], F16, name=f"pth{j}", tag=f"pth{j}", bufs=1)
            for j in range(8)
        ]
        with tc.tile_pool(name="pp", bufs=2, space="PSUM") as pp:
            proj_wave(0, pp)
            proj_wave(1, pp)
            # early QK+exp for (hp0, ic0) j-tiles 0-7 overlap the remaining
            # projection waves; their PV runs later (accumulator banks are
            # still occupied by the projection pool here).
            for jt in range(8):
                qk_exp(0, 0, jt, pt_hold[jt])
            proj_wave(2, pp)
            proj_wave(3, pp)

    # --- attention ---
    with tc.tile_pool(name="op", bufs=2, space="PSUM") as opool:
        for hp in range(2):
            for ic in range(NI):
                o_ps = [
                    opool.tile(
                        [DH + 1, IC], F32, name=f"ops{i}", tag=f"ops{i}", bufs=2
                    )
                    for i in range(2)
                ]
                if hp == 0 and ic == 0:
                    for jt in range(8):
                        pv_mm(o_ps, jt, pt_hold[jt])
                    attn_block(hp, ic, o_ps, range(8, NJ))
                else:
                    attn_block(hp, ic, o_ps, range(NJ))
                drain_block(hp, ic, o_ps)


_CACHE = {}


def build():
    if "nc" in _CACHE:
        return _CACHE["nc"]
    nc = bacc.Bacc(
        "TRN2", target_bir_lowering=False, debug=False, num_devices=NCORES
    )
    xT = nc.dram_tensor("xT", (DIM, N), F16, kind="ExternalInput").ap()
    wq = nc.dram_tensor("wq", (DIM, G * DH), F16, kind="ExternalInput").ap()
    wk = nc.dram_tensor("wk", (DIM, G * DH), F16, kind="ExternalInput").ap()
    wv = nc.dram_tensor("wv", (DIM, DH), F16, kind="ExternalInput").ap()
    eye = nc.dram_tensor("eye", (DH, DH), F16, kind="ExternalInput").ap()
    oT = nc.dram_tensor("oT", (G * DH, N), F32, kind="ExternalOutput").ap()
    with tile.TileContext(nc) as tc:
        with ExitStack() as ctx:
            emit_kernel(ctx, tc, xT, wq, wk, wv, eye, oT)
    nc.compile()
    _CACHE["nc"] = nc
    return nc


def make_in_maps(x, Wq, Wkv, k_scale):
    x = np.asarray(x, dtype=np.float32)
    Wq = np.asarray(Wq, dtype=np.float32)
    Wkv = np.asarray(Wkv, dtype=np.float32)
    k_scale = np.asarray(k_scale, dtype=np.float32)
    xTs = [np.ascontiguousarray(x[b].T) for b in range(B)]
    in_maps = []
    for c in range(NCORES):
        b, kv = divmod(c, KV_HEADS)
        wk_base = Wkv[:, kv * DH : (kv + 1) * DH]
        wk_c = np.concatenate(
            [wk_base * k_scale[kv * G + j, 0][None, :] for j in range(G)], axis=1
        )
        in_maps.append(
            {
                "xT": xTs[b].astype(np.float16),
                "wq": np.ascontiguousarray(Wq[:, kv * G * DH : (kv + 1) * G * DH]).astype(np.float16),
                "wk": np.ascontiguousarray(wk_c).astype(np.float16),
                "wv": np.ascontiguousarray(
                    Wkv[:, KV_HEADS * DH + kv * DH : KV_HEADS * DH + (kv + 1) * DH]
                ).astype(np.float16),
                "eye": np.eye(DH, dtype=np.float16),
            }
        )
    return in_maps


def gather(results):
    out = np.empty((B, N, HEADS * DH), dtype=np.float32)
    for c in range(NCORES):
        b, kv = divmod(c, KV_HEADS)
        out[b, :, kv * G * DH : (kv + 1) * G * DH] = results[c]["oT"].T
    return out


def kernel(x, Wq, Wkv, k_scale, _trace=False):
    nc = build()
    in_maps = make_in_maps(x, Wq, Wkv, k_scale)
    res = run_bass_kernel_spmd(
        nc, in_maps, core_ids=list(range(NCORES)), trace=_trace
    )
    out = gather(res.results)
    if _trace:
        kernel.last_result = res
    return out



# revision 29
# speedup vs baseline: 1.0660x; 1.0439x over previous
"""Trainium2 Bass kernel for grouped-query attention with qk-norm.

Problem (hardcoded): x(2,2048,1024) @ Wq(1024,1024) / Wkv(1024,512),
16 query heads, 4 kv heads, head_dim 64, k_scale(16,1,64) applied to the
group-broadcast k. Output (2,2048,1024).

Sharding: 8 cores = batch(2) x kv_heads(4). Each core computes its batch's
4 query heads against its kv head over the full 2048x2048 score matrix.

Device kernel layout choices:
- Host passes x transposed and kt-tiled (dim on partitions) so projection
  matmuls contract over dim with no on-device transposes, and all weights
  pre-tiled to [128, kt-major] so every input DMA is contiguous 1-8KB lines.
- k_scale is folded into Wq host-side ((q*ks)@k^T == q@(k*ks)^T), so k is
  projected once per kv head (64 wide) and shared by all 4 query heads;
  the k and v projections are column-tiled on the PE (both M=64) and run
  concurrently. kT is duplicated to partitions 64-127 by an SBUF-to-SBUF
  DMA so both QK row-tiles have a stationary copy.
- Scores are computed transposed (S^T: keys on partitions, queries free)
  so that exp(S^T) tiles feed the PV matmul directly as the moving
  operand (no P transpose). All matmul inputs are fp16 (1 row/cycle).
- Softmax skips the max-subtraction (inputs are bounded; exp stays well
  inside fp32 range) and normalizes after PV via an appended ones-row in
  the V stationary operand (row 64 of the PV psum accumulates sum(exp)).
- The first two attention blocks are interleaved with the projection
  chains so the ScalarE exp stream (the pipeline limiter) starts as soon
  as the first kv chunk is projected.
- Output is returned transposed per head (oT: 4*64 x 2048); the host
  transposes during the gather.
"""

from contextlib import ExitStack

import numpy as np

import concourse.bacc as bacc
import concourse.mybir as mybir
import concourse.tile as tile
from concourse.bass_utils import run_bass_kernel_spmd

# Problem constants
B, N, DIM = 2, 2048, 1024
HEADS, KV_HEADS, DH = 16, 4, 64
G = HEADS // KV_HEADS  # query heads per kv head (4)
NCORES = 8
P = 128
KT = DIM // P  # 8 contraction tiles over dim
IC = 512  # query-chunk width
NI = N // IC  # 4
NJ = N // P  # 16 key tiles
SCALE = DH**-0.5

F32 = mybir.dt.float32
F16 = mybir.dt.float16

DEBUG_DUMP = False


def emit_kernel(ctx, tc, xt, wq, wk, wv, eye, oT):
    nc = tc.nc
    Exp = mybir.ActivationFunctionType.Exp
    mult = mybir.AluOpType.mult

    wpool = ctx.enter_context(tc.tile_pool(name="w", bufs=1))
    qkpool = ctx.enter_context(tc.tile_pool(name="qk", bufs=1))
    ptpool = ctx.enter_context(tc.tile_pool(name="pt", bufs=6))
    npool = ctx.enter_context(tc.tile_pool(name="norm", bufs=2))

    # --- persistent SBUF tensors ---
    ones_sb = wpool.tile([P, DH], F16, tag="ones")
    eye_sb = wpool.tile([DH, DH], F16, tag="eye")
    wq_sb = wpool.tile([P, KT * 256], F16, tag="wq")
    wk_sb = wpool.tile([P, KT * DH], F16, tag="wk")
    wv_sb = wpool.tile([P, KT * DH], F16, tag="wv")
    xts = wpool.tile([P, KT * N], F16, tag="xt")  # 4MB, [p, (ic, kt, c)]
    qT = [qkpool.tile([P, N], F16, name=f"qT{hp}", tag=f"qT{hp}") for hp in range(2)]
    kkT = qkpool.tile([P, N], F16, tag="kkT")  # kT on both partition halves
    vT_sb = qkpool.tile([DH, N], F16, tag="vT")
    vaug = qkpool.tile([P, NJ * (DH + 1)], F16, tag="vaug")
    nc.any.memset(vaug[:], 1.0)
    nc.any.memset(ones_sb[:], 1.0)
    warm = qkpool.tile([1, 2], F32, tag="warm")
    nc.any.memset(warm[0:1, 0:1], 0.0)
    nc.scalar.activation(warm[0:1, 1:2], warm[0:1, 0:1], Exp)

    # --- input DMAs: weights first (k/v first -- the kv chain starts the
    # ramp), x in contiguous half-ic chunks, over the Sync/GpSimd queues ---
    nc.sync.dma_start(wk_sb[:], wk[:, :])
    nc.sync.dma_start(wv_sb[:], wv[:, :])
    nc.sync.dma_start(eye_sb[:], eye[:, :])
    nc.gpsimd.dma_start(wq_sb[:], wq[:, :])
    XW = KT * IC  # columns per ic-chunk of xts
    for ic in range(NI):
        for h in range(2):
            eng = nc.sync if (2 * ic + h) % 2 == 0 else nc.gpsimd
            c0 = ic * XW + h * (XW // 2)
            eng.dma_start(xts[:, c0 : c0 + XW // 2], xt[:, c0 : c0 + XW // 2])

    kdup = nc.dram_tensor("kdup", (DH, N), F16, kind="ExternalOutput").ap()

    # --- psum pools (8 banks total: st 4 + proj 2 + PV accumulators 2) ---
    apsum = ctx.enter_context(tc.tile_pool(name="ap", bufs=2, space="PSUM"))
    pp = ctx.enter_context(tc.tile_pool(name="pp", bufs=2, space="PSUM"))
    opool = ctx.enter_context(tc.tile_pool(name="op", bufs=1, space="PSUM"))

    # Dummy matmuls during the initial DMA wait keep the PE HAM activity
    # monitor busy so real projections start at 2.4GHz instead of 1.2.
    for _ in range(28):
        wt = apsum.tile([DH, IC], F32, tag="s", name="wt", bufs=2)
        nc.tensor.matmul(
            wt[:, 0:DH], ones_sb[:, 0:DH], ones_sb[:, 0:DH],
            start=True, stop=True,
        )

    # --- projection chains ---
    def q_chain(hp, ic):
        csl = slice(ic * IC, (ic + 1) * IC)
        ps = pp.tile([P, IC], F32, tag="pj", name="pjq", bufs=2)
        for kt in range(KT):
            c0 = kt * 256 + hp * 128
            nc.tensor.matmul(
                ps[:],
                wq_sb[:, c0 : c0 + 128],
                xts[:, (ic * KT + kt) * IC : (ic * KT + kt + 1) * IC],
                start=(kt == 0),
                stop=(kt == KT - 1),
            )
        nc.vector.tensor_copy(qT[hp][:, csl], ps[:])

    def kv_mms(ic):
        # k and v projections (both M=64; PE col-tiling them concurrently
        # is not an option -- the high col-group quadrant has a HW bug)
        csl = slice(ic * IC, (ic + 1) * IC)
        ps_v = pp.tile([P, IC], F32, tag="pj", name="pjv", bufs=2)
        ps_k = pp.tile([P, IC], F32, tag="pj", name="pjk", bufs=2)
        for kt in range(KT):
            xs = xts[:, (ic * KT + kt) * IC : (ic * KT + kt + 1) * IC]
            nc.tensor.matmul(
                ps_v[0:DH, :],
                wv_sb[:, kt * DH : (kt + 1) * DH],
                xs,
                start=(kt == 0),
                stop=(kt == KT - 1),
            )
        for kt in range(KT):
            xs = xts[:, (ic * KT + kt) * IC : (ic * KT + kt + 1) * IC]
            nc.tensor.matmul(
                ps_k[0:DH, :],
                wk_sb[:, kt * DH : (kt + 1) * DH],
                xs,
                start=(kt == 0),
                stop=(kt == KT - 1),
            )
        nc.vector.tensor_copy(vT_sb[:, csl], ps_v[0:DH, :])
        nc.vector.tensor_copy(kkT[0:DH, csl], ps_k[0:DH, :])
        # duplicate kT to partitions 64-127 for the second QK row-tile via
        # a DRAM bounce (both on the sync queue, which executes in order)
        nc.sync.dma_start(kdup[:, csl], kkT[0:DH, csl])
        nc.sync.dma_start(kkT[DH:P, csl], kdup[:, csl])

    def kv_tail(ic):
        # vaug tiles (v transposed, with the ones-row kept from the memset)
        for jt in range(4 * ic, 4 * ic + 4):
            pv = pp.tile([P, DH], F16, tag="pj", bufs=2, name="pvt")
            nc.tensor.transpose(pv[:], vT_sb[:, jt * P : (jt + 1) * P], eye_sb[:])
            nc.vector.tensor_copy(
                vaug[:, jt * (DH + 1) : jt * (DH + 1) + DH], pv[:]
            )

    # --- attention ---
    def qk_exp(hp, ic, jt, pt):
        csl = slice(ic * IC, (ic + 1) * IC)
        st = apsum.tile([P, 2 * IC], F32, tag="s", bufs=2, name="st")
        for half in range(2):
            rsl = slice(half * 64, half * 64 + 64)
            nc.tensor.matmul(
                st[:, half * IC : (half + 1) * IC],
                kkT[rsl, jt * P : (jt + 1) * P],
                qT[hp][rsl, csl],
                start=True,
                stop=True,
                tile_position=(half * 64, 0),
            )
        nc.scalar.activation(pt[:], st[:], Exp, scale=SCALE)

    def pv_mm(o_ps, jt, pt):
        for half in range(2):
            nc.tensor.matmul(
                o_ps[half][:],
                vaug[:, jt * (DH + 1) : (jt + 1) * (DH + 1)],
                pt[:, half * IC : (half + 1) * IC],
                start=(jt == 0),
                stop=(jt == NJ - 1),
            )

    def attn4(hp, ic, o_ps, j0):
        pts = []
        for jt in range(j0, j0 + 4):
            pt = ptpool.tile([P, 2 * IC], F16, tag="pt")
            qk_exp(hp, ic, jt, pt)
            pts.append(pt)
        return pts

    def pv4(o_ps, j0, pts):
        for jt, pt in zip(range(j0, j0 + 4), pts):
            pv_mm(o_ps, jt, pt)

    def new_ops():
        return [
            opool.tile([DH + 1, IC], F32, name=f"ops{i}", tag=f"ops{i}", bufs=1)
            for i in range(2)
        ]

    def normalize_half(h, ic, fo):
        # GpSimd broadcasts the sums row across partitions (PE-free), then
        # a single-pass approx reciprocal (~18 bits) and the final multiply.
        # partition_broadcast needs its source on partition 0 (HW reads
        # channel 0 regardless of the AP base), so stage the row first.
        csl = slice(ic * IC, (ic + 1) * IC)
        srow = npool.tile([1, IC], F32, tag="srow", bufs=4)
        nc.vector.tensor_copy(srow[:], fo[DH : DH + 1, :])
        bc = npool.tile([DH, IC], F32, name="bcg", tag="bcg", bufs=4)
        nc.gpsimd.partition_broadcast(bc[:], srow[:])
        rb = npool.tile([DH, IC], F32, tag="rb", bufs=4)
        nc.vector.reciprocal_approx_fast(rb[:], bc[:])
        fin = npool.tile([DH, IC], F32, tag="fin", bufs=4)
        nc.vector.tensor_tensor(fin[:], fo[0:DH, :], rb[:], mult)
        nc.sync.dma_start(oT[h * DH : (h + 1) * DH, csl], fin[:])

    def drain_block(hp, ic, o_ps):
        # copy out of PSUM promptly so the next block's PV can start
        for half in range(2):
            fo = npool.tile([DH + 1, IC], F32, tag="fo", bufs=2, name="fo")
            nc.vector.tensor_copy(fo[:], o_ps[half][:])
            normalize_half(2 * hp + half, ic, fo)

    # --- ramp: first two attention blocks interleaved with projections ---
    kv_mms(0)
    q_chain(0, 0)
    kv_tail(0)
    o_ps = new_ops()
    pts = attn4(0, 0, o_ps, 0)
    q_chain(1, 0)
    pv4(o_ps, 0, pts)
    kv_mms(1)
    pts = attn4(0, 0, o_ps, 4)
    kv_tail(1)
    q_chain(0, 1)
    pv4(o_ps, 4, pts)
    kv_mms(2)
    pts = attn4(0, 0, o_ps, 8)
    kv_tail(2)
    q_chain(1, 1)
    pv4(o_ps, 8, pts)
    kv_mms(3)
    pts = attn4(0, 0, o_ps, 12)
    kv_tail(3)
    q_chain(0, 2)
    pv4(o_ps, 12, pts)
    drain_block(0, 0, o_ps)

    # block (1, 0) interleaved with the remaining q chains
    o_ps = new_ops()
    pts = attn4(1, 0, o_ps, 0)
    q_chain(1, 2)
    pv4(o_ps, 0, pts)
    pts = attn4(1, 0, o_ps, 4)
    q_chain(0, 3)
    pv4(o_ps, 4, pts)
    pts = attn4(1, 0, o_ps, 8)
    q_chain(1, 3)
    pv4(o_ps, 8, pts)
    pts = attn4(1, 0, o_ps, 12)
    pv4(o_ps, 12, pts)
    drain_block(1, 0, o_ps)

    # --- steady-state blocks ---
    for ic in range(1, NI):
        for hp in range(2):
            o_ps = new_ops()
            for j0 in range(0, NJ, 4):
                pts = attn4(hp, ic, o_ps, j0)
                pv4(o_ps, j0, pts)
            drain_block(hp, ic, o_ps)

    if DEBUG_DUMP:
        for name, t, shape in [
            ("dbg_wq", wq_sb, (P, KT * 256)), ("dbg_wk", wk_sb, (P, KT * DH)),
            ("dbg_wv", wv_sb, (P, KT * DH)), ("dbg_kkT", kkT, (P, N)),
            ("dbg_qT0", qT[0], (P, N)), ("dbg_qT1", qT[1], (P, N)),
            ("dbg_vT", vT_sb, (DH, N)), ("dbg_vaug", vaug, (P, NJ * (DH + 1))),
            ("dbg_xts", xts, (P, KT * N)),
        ]:
            d = nc.dram_tensor(name, shape, F16, kind="ExternalOutput").ap()
            nc.sync.dma_start(d[:, :], t[:])


_CACHE = {}


def build():
    if "nc" in _CACHE:
        return _CACHE["nc"]
    nc = bacc.Bacc(
        "TRN2", target_bir_lowering=False, debug=False, num_devices=NCORES
    )
    xt = nc.dram_tensor("xt", (P, KT * N), F16, kind="ExternalInput").ap()
    wq = nc.dram_tensor("wq", (P, KT * 256), F16, kind="ExternalInput").ap()
    wk = nc.dram_tensor("wk", (P, KT * DH), F16, kind="ExternalInput").ap()
    wv = nc.dram_tensor("wv", (P, KT * DH), F16, kind="ExternalInput").ap()
    eye = nc.dram_tensor("eye", (DH, DH), F16, kind="ExternalInput").ap()
    oT = nc.dram_tensor("oT", (G * DH, N), F32, kind="ExternalOutput").ap()
    with tile.TileContext(nc) as tc:
        with ExitStack() as ctx:
            emit_kernel(ctx, tc, xt, wq, wk, wv, eye, oT)
    nc.compile()
    _CACHE["nc"] = nc
    return nc


def _tile_kt(w):
    # (1024, C) -> (128, KT*C): row-block kt lands at column block kt
    C = w.shape[1]
    return np.ascontiguousarray(
        w.reshape(KT, P, C).transpose(1, 0, 2).reshape(P, KT * C)
    )


def make_in_maps(x, Wq, Wkv, k_scale):
    x = np.asarray(x, dtype=np.float32)
    Wq = np.asarray(Wq, dtype=np.float32)
    Wkv = np.asarray(Wkv, dtype=np.float32)
    k_scale = np.asarray(k_scale, dtype=np.float32)
    # x[b].T tiled to [p, (ic, kt, c)] so each ic-chunk is one contiguous DMA
    xts = []
    for b in range(B):
        xT = x[b].T.reshape(KT, P, NI, IC)
        xts.append(
            np.ascontiguousarray(xT.transpose(1, 2, 0, 3).reshape(P, KT * N)).astype(
                np.float16
            )
        )
    in_maps = []
    for c in range(NCORES):
        b, kv = divmod(c, KV_HEADS)
        # fold the per-query-head k_scale into Wq: (q*ks)@k^T == q@(k*ks)^T
        wq_c = np.concatenate(
            [
                Wq[:, (kv * G + j) * DH : (kv * G + j + 1) * DH]
                * k_scale[kv * G + j, 0][None, :]
                for j in range(G)
            ],
            axis=1,
        )
        wk_c = Wkv[:, kv * DH : (kv + 1) * DH]
        wv_c = Wkv[:, KV_HEADS * DH + kv * DH : KV_HEADS * DH + (kv + 1) * DH]
        in_maps.append(
            {
                "xt": xts[b],
                "wq": _tile_kt(wq_c).astype(np.float16),
                "wk": _tile_kt(wk_c).astype(np.float16),
                "wv": _tile_kt(wv_c).astype(np.float16),
                "eye": np.eye(DH, dtype=np.float16),
            }
        )
    return in_maps


def gather(results):
    out = np.empty((B, N, HEADS * DH), dtype=np.float32)
    for c in range(NCORES):
        b, kv = divmod(c, KV_HEADS)
        out[b, :, kv * G * DH : (kv + 1) * G * DH] = results[c]["oT"].T
    return out


def kernel(x, Wq, Wkv, k_scale, _trace=False):
    nc = build()
    in_maps = make_in_maps(x, Wq, Wkv, k_scale)
    res = run_bass_kernel_spmd(
        nc, in_maps, core_ids=list(range(NCORES)), trace=_trace
    )
    out = gather(res.results)
    if _trace:
        kernel.last_result = res
    return out


# revision 36
# speedup vs baseline: 1.0861x; 1.0189x over previous
"""Trainium2 Bass kernel for grouped-query attention with qk-norm.

Problem (hardcoded): x(2,2048,1024) @ Wq(1024,1024) / Wkv(1024,512),
16 query heads, 4 kv heads, head_dim 64, k_scale(16,1,64) applied to the
group-broadcast k. Output (2,2048,1024).

Sharding: 8 cores = batch(2) x kv_heads(4). Each core computes its batch's
4 query heads against its kv head over the full 2048x2048 score matrix.

Device kernel layout choices:
- Host passes x transposed and kt-tiled (dim on partitions) so projection
  matmuls contract over dim with no on-device transposes, and all weights
  pre-tiled to [128, kt-major] so every input DMA is contiguous 1-8KB lines.
- k_scale is folded into Wq host-side ((q*ks)@k^T == q@(k*ks)^T), so k is
  projected once per kv head (64 wide) and shared by all 4 query heads;
  the k and v projections are column-tiled on the PE (both M=64) and run
  concurrently. kT is duplicated to partitions 64-127 by an SBUF-to-SBUF
  DMA so both QK row-tiles have a stationary copy.
- Scores are computed transposed (S^T: keys on partitions, queries free)
  so that exp(S^T) tiles feed the PV matmul directly as the moving
  operand (no P transpose). All matmul inputs are fp16 (1 row/cycle).
- Softmax skips the max-subtraction (inputs are bounded; exp stays well
  inside fp32 range) and normalizes after PV via an appended ones-row in
  the V stationary operand (row 64 of the PV psum accumulates sum(exp)).
- The first two attention blocks are interleaved with the projection
  chains so the ScalarE exp stream (the pipeline limiter) starts as soon
  as the first kv chunk is projected.
- Output is returned transposed per head (oT: 4*64 x 2048); the host
  transposes during the gather.
"""

from contextlib import ExitStack

import numpy as np

import concourse.bacc as bacc
import concourse.mybir as mybir
import concourse.tile as tile
from concourse.bass_utils import run_bass_kernel_spmd

# Problem constants
B, N, DIM = 2, 2048, 1024
HEADS, KV_HEADS, DH = 16, 4, 64
G = HEADS // KV_HEADS  # query heads per kv head (4)
NCORES = 8
P = 128
KT = DIM // P  # 8 contraction tiles over dim
IC = 512  # query-chunk width
NI = N // IC  # 4
NJ = N // P  # 16 key tiles
SCALE = DH**-0.5

F32 = mybir.dt.float32
F16 = mybir.dt.float16

DEBUG_DUMP = False


def emit_kernel(ctx, tc, xt, wq, wk, wv, eye, oT):
    nc = tc.nc
    Exp = mybir.ActivationFunctionType.Exp
    mult = mybir.AluOpType.mult

    wpool = ctx.enter_context(tc.tile_pool(name="w", bufs=1))
    qkpool = ctx.enter_context(tc.tile_pool(name="qk", bufs=1))
    ptpool = ctx.enter_context(tc.tile_pool(name="pt", bufs=6))
    npool = ctx.enter_context(tc.tile_pool(name="norm", bufs=2))

    # --- persistent SBUF tensors ---
    ones_sb = wpool.tile([P, DH], F16, tag="ones")
    eye_sb = wpool.tile([DH, DH], F16, tag="eye")
    wq_sb = wpool.tile([P, KT * 256], F16, tag="wq")
    wk_sb = wpool.tile([P, KT * 128], F16, tag="wk")  # [wk|wk] duplicated
    wv_sb = wpool.tile([P, KT * DH], F16, tag="wv")
    xts = wpool.tile([P, KT * N], F16, tag="xt")  # 4MB, [p, (ic, kt, c)]
    qT = [qkpool.tile([P, N], F16, name=f"qT{hp}", tag=f"qT{hp}") for hp in range(2)]
    kkT = qkpool.tile([P, N], F16, tag="kkT")  # kT on both partition halves
    vT_sb = qkpool.tile([DH, N], F16, tag="vT")
    vaug = qkpool.tile([P, NJ * (DH + 1)], F16, tag="vaug")
    nc.any.memset(vaug[:], 1.0)
    nc.any.memset(ones_sb[:], 1.0)
    warm = qkpool.tile([1, 2], F32, tag="warm")
    nc.any.memset(warm[0:1, 0:1], 0.0)
    nc.scalar.activation(warm[0:1, 1:2], warm[0:1, 0:1], Exp)

    # --- input DMAs: weights first (k/v first -- the kv chain starts the
    # ramp); x chunks are emitted lazily in the ramp schedule below so each
    # queue's FIFO matches the order the data is needed ---
    nc.sync.dma_start(wk_sb[:], wk[:, :])
    nc.sync.dma_start(wv_sb[:], wv[:, :])
    nc.sync.dma_start(eye_sb[:], eye[:, :])
    nc.gpsimd.dma_start(wq_sb[:], wq[:, :])
    XW = KT * IC  # columns per ic-chunk of xts

    def dma_x(ic):
        for h in range(2):
            eng = nc.sync if h == 0 else nc.gpsimd
            c0 = ic * XW + h * (XW // 2)
            eng.dma_start(xts[:, c0 : c0 + XW // 2], xt[:, c0 : c0 + XW // 2])

    dma_x(0)

    # --- psum pools (8 banks total: st 4 + proj 2 + PV accumulators 2) ---
    apsum = ctx.enter_context(tc.tile_pool(name="ap", bufs=2, space="PSUM"))
    pp = ctx.enter_context(tc.tile_pool(name="pp", bufs=2, space="PSUM"))
    opool = ctx.enter_context(tc.tile_pool(name="op", bufs=1, space="PSUM"))

    # Dummy matmuls during the initial DMA wait keep the PE HAM activity
    # monitor busy so real projections start at 2.4GHz instead of 1.2.
    for _ in range(40):
        wt = apsum.tile([DH, IC], F32, tag="s", name="wt", bufs=2)
        nc.tensor.matmul(
            wt[:, 0:DH], ones_sb[:, 0:DH], ones_sb[:, 0:DH],
            start=True, stop=True,
        )

    # --- projection chains ---
    def q_chain(hp, ic):
        csl = slice(ic * IC, (ic + 1) * IC)
        ps = pp.tile([P, IC], F32, tag="pj", name="pjq", bufs=2)
        for kt in range(KT):
            c0 = kt * 256 + hp * 128
            nc.tensor.matmul(
                ps[:],
                wq_sb[:, c0 : c0 + 128],
                xts[:, (ic * KT + kt) * IC : (ic * KT + kt + 1) * IC],
                start=(kt == 0),
                stop=(kt == KT - 1),
            )
        nc.vector.tensor_copy(qT[hp][:, csl], ps[:])

    def kv_mms(ic):
        # k projection with host-duplicated [wk|wk] stationary (M=128, same
        # cycles as M=64): psum rows 0-63 AND 64-127 both get kT, so one
        # copy fills both QK row-tile stationary halves. v separate (M=64).
        csl = slice(ic * IC, (ic + 1) * IC)
        ps_k = pp.tile([P, IC], F32, tag="pj", name="pjk", bufs=2)
        ps_v = pp.tile([P, IC], F32, tag="pj", name="pjv", bufs=2)
        for kt in range(KT):
            xs = xts[:, (ic * KT + kt) * IC : (ic * KT + kt + 1) * IC]
            nc.tensor.matmul(
                ps_k[:, :],
                wk_sb[:, kt * 128 : (kt + 1) * 128],
                xs,
                start=(kt == 0),
                stop=(kt == KT - 1),
            )
        for kt in range(KT):
            xs = xts[:, (ic * KT + kt) * IC : (ic * KT + kt + 1) * IC]
            nc.tensor.matmul(
                ps_v[0:DH, :],
                wv_sb[:, kt * DH : (kt + 1) * DH],
                xs,
                start=(kt == 0),
                stop=(kt == KT - 1),
            )
        nc.vector.tensor_copy(kkT[:, csl], ps_k[:, :])
        nc.vector.tensor_copy(vT_sb[:, csl], ps_v[0:DH, :])

    def kv_tail(ic):
        # vaug tiles (v transposed, with the ones-row kept from the memset)
        for jt in range(4 * ic, 4 * ic + 4):
            pv = pp.tile([P, DH], F16, tag="pj", bufs=2, name="pvt")
            nc.tensor.transpose(pv[:], vT_sb[:, jt * P : (jt + 1) * P], eye_sb[:])
            nc.vector.tensor_copy(
                vaug[:, jt * (DH + 1) : jt * (DH + 1) + DH], pv[:]
            )

    # --- attention ---
    def qk_exp(hp, ic, jt, pt):
        csl = slice(ic * IC, (ic + 1) * IC)
        st = apsum.tile([P, 2 * IC], F32, tag="s", bufs=2, name="st")
        for half in range(2):
            rsl = slice(half * 64, half * 64 + 64)
            nc.tensor.matmul(
                st[:, half * IC : (half + 1) * IC],
                kkT[rsl, jt * P : (jt + 1) * P],
                qT[hp][rsl, csl],
                start=True,
                stop=True,
                tile_position=(half * 64, 0),
            )
        nc.scalar.activation(pt[:], st[:], Exp, scale=SCALE)

    def pv_mm(o_ps, jt, pt):
        for half in range(2):
            nc.tensor.matmul(
                o_ps[half][:],
                vaug[:, jt * (DH + 1) : (jt + 1) * (DH + 1)],
                pt[:, half * IC : (half + 1) * IC],
                start=(jt == 0),
                stop=(jt == NJ - 1),
            )

    def attn4(hp, ic, o_ps, j0):
        pts = []
        for jt in range(j0, j0 + 4):
            pt = ptpool.tile([P, 2 * IC], F16, tag="pt")
            qk_exp(hp, ic, jt, pt)
            pts.append(pt)
        return pts

    def pv4(o_ps, j0, pts):
        for jt, pt in zip(range(j0, j0 + 4), pts):
            pv_mm(o_ps, jt, pt)

    def new_ops():
        return [
            opool.tile([DH + 1, IC], F32, name=f"ops{i}", tag=f"ops{i}", bufs=1)
            for i in range(2)
        ]

    def normalize_half(h, ic, fo):
        # GpSimd broadcasts the sums row across partitions (PE-free), then
        # a single-pass approx reciprocal (~18 bits) and the final multiply.
        # partition_broadcast needs its source on partition 0 (HW reads
        # channel 0 regardless of the AP base), so stage the row first.
        csl = slice(ic * IC, (ic + 1) * IC)
        srow = npool.tile([1, IC], F32, tag="srow", bufs=4)
        nc.vector.tensor_copy(srow[:], fo[DH : DH + 1, :])
        bc = npool.tile([DH, IC], F32, name="bcg", tag="bcg", bufs=4)
        nc.gpsimd.partition_broadcast(bc[:], srow[:])
        rb = npool.tile([DH, IC], F32, tag="rb", bufs=4)
        nc.vector.reciprocal_approx_fast(rb[:], bc[:])
        fin = npool.tile([DH, IC], F32, tag="fin", bufs=4)
        nc.vector.tensor_tensor(fin[:], fo[0:DH, :], rb[:], mult)
        nc.sync.dma_start(oT[h * DH : (h + 1) * DH, csl], fin[:])

    def drain_block(hp, ic, o_ps):
        # copy out of PSUM promptly so the next block's PV can start
        for half in range(2):
            fo = npool.tile([DH + 1, IC], F32, tag="fo", bufs=2, name="fo")
            nc.vector.tensor_copy(fo[:], o_ps[half][:])
            normalize_half(2 * hp + half, ic, fo)

    # --- ramp: first two attention blocks interleaved with projections ---
    kv_mms(0)
    q_chain(0, 0)
    dma_x(1)
    kv_tail(0)
    o_ps = new_ops()
    pts = attn4(0, 0, o_ps, 0)
    dma_x(2)
    q_chain(1, 0)
    pv4(o_ps, 0, pts)
    kv_mms(1)
    pts = attn4(0, 0, o_ps, 4)
    dma_x(3)
    kv_tail(1)
    q_chain(0, 1)
    pv4(o_ps, 4, pts)
    kv_mms(2)
    pts = attn4(0, 0, o_ps, 8)
    kv_tail(2)
    q_chain(1, 1)
    pv4(o_ps, 8, pts)
    kv_mms(3)
    pts = attn4(0, 0, o_ps, 12)
    kv_tail(3)
    q_chain(0, 2)
    pv4(o_ps, 12, pts)
    drain_block(0, 0, o_ps)

    # block (1, 0) interleaved with the remaining q chains
    o_ps = new_ops()
    pts = attn4(1, 0, o_ps, 0)
    q_chain(1, 2)
    pv4(o_ps, 0, pts)
    pts = attn4(1, 0, o_ps, 4)
    q_chain(0, 3)
    pv4(o_ps, 4, pts)
    pts = attn4(1, 0, o_ps, 8)
    q_chain(1, 3)
    pv4(o_ps, 8, pts)
    pts = attn4(1, 0, o_ps, 12)
    pv4(o_ps, 12, pts)
    drain_block(1, 0, o_ps)

    # --- steady-state blocks ---
    for ic in range(1, NI):
        for hp in range(2):
            o_ps = new_ops()
            for j0 in range(0, NJ, 4):
                pts = attn4(hp, ic, o_ps, j0)
                pv4(o_ps, j0, pts)
            drain_block(hp, ic, o_ps)

    if DEBUG_DUMP:
        for name, t, shape in [
            ("dbg_wq", wq_sb, (P, KT * 256)), ("dbg_wk", wk_sb, (P, KT * DH)),
            ("dbg_wv", wv_sb, (P, KT * DH)), ("dbg_kkT", kkT, (P, N)),
            ("dbg_qT0", qT[0], (P, N)), ("dbg_qT1", qT[1], (P, N)),
            ("dbg_vT", vT_sb, (DH, N)), ("dbg_vaug", vaug, (P, NJ * (DH + 1))),
            ("dbg_xts", xts, (P, KT * N)),
        ]:
            d = nc.dram_tensor(name, shape, F16, kind="ExternalOutput").ap()
            nc.sync.dma_start(d[:, :], t[:])


_CACHE = {}


def build():
    if "nc" in _CACHE:
        return _CACHE["nc"]
    nc = bacc.Bacc(
        "TRN2", target_bir_lowering=False, debug=False, num_devices=NCORES
    )
    xt = nc.dram_tensor("xt", (P, KT * N), F16, kind="ExternalInput").ap()
    wq = nc.dram_tensor("wq", (P, KT * 256), F16, kind="ExternalInput").ap()
    wk = nc.dram_tensor("wk", (P, KT * 128), F16, kind="ExternalInput").ap()
    wv = nc.dram_tensor("wv", (P, KT * DH), F16, kind="ExternalInput").ap()
    eye = nc.dram_tensor("eye", (DH, DH), F16, kind="ExternalInput").ap()
    oT = nc.dram_tensor("oT", (G * DH, N), F32, kind="ExternalOutput").ap()
    with tile.TileContext(nc) as tc:
        with ExitStack() as ctx:
            emit_kernel(ctx, tc, xt, wq, wk, wv, eye, oT)
    nc.compile()
    _CACHE["nc"] = nc
    return nc


def _tile_kt(w):
    # (1024, C) -> (128, KT*C): row-block kt lands at column block kt
    C = w.shape[1]
    return np.ascontiguousarray(
        w.reshape(KT, P, C).transpose(1, 0, 2).reshape(P, KT * C)
    )


def make_in_maps(x, Wq, Wkv, k_scale):
    x = np.asarray(x, dtype=np.float32)
    Wq = np.asarray(Wq, dtype=np.float32)
    Wkv = np.asarray(Wkv, dtype=np.float32)
    k_scale = np.asarray(k_scale, dtype=np.float32)
    # x[b].T tiled to [p, (ic, kt, c)] so each ic-chunk is one contiguous DMA
    xts = []
    for b in range(B):
        xT = x[b].T.reshape(KT, P, NI, IC)
        xts.append(
            np.ascontiguousarray(xT.transpose(1, 2, 0, 3).reshape(P, KT * N)).astype(
                np.float16
            )
        )
    in_maps = []
    for c in range(NCORES):
        b, kv = divmod(c, KV_HEADS)
        # fold the per-query-head k_scale into Wq: (q*ks)@k^T == q@(k*ks)^T
        wq_c = np.concatenate(
            [
                Wq[:, (kv * G + j) * DH : (kv * G + j + 1) * DH]
                * k_scale[kv * G + j, 0][None, :]
                for j in range(G)
            ],
            axis=1,
        )
        wk_c = Wkv[:, kv * DH : (kv + 1) * DH]
        wv_c = Wkv[:, KV_HEADS * DH + kv * DH : KV_HEADS * DH + (kv + 1) * DH]
        in_maps.append(
            {
                "xt": xts[b],
                "wq": _tile_kt(wq_c).astype(np.float16),
                "wk": _tile_kt(np.concatenate([wk_c, wk_c], axis=1)).astype(
                    np.float16
                ),
                "wv": _tile_kt(wv_c).astype(np.float16),
                "eye": np.eye(DH, dtype=np.float16),
            }
        )
    return in_maps


def gather(results):
    out = np.empty((B, N, HEADS * DH), dtype=np.float32)
    for c in range(NCORES):
        b, kv = divmod(c, KV_HEADS)
        out[b, :, kv * G * DH : (kv + 1) * G * DH] = results[c]["oT"].T
    return out


def kernel(x, Wq, Wkv, k_scale, _trace=False):
    nc = build()
    in_maps = make_in_maps(x, Wq, Wkv, k_scale)
    res = run_bass_kernel_spmd(
        nc, in_maps, core_ids=list(range(NCORES)), trace=_trace
    )
    out = gather(res.results)
    if _trace:
        kernel.last_result = res
    return out


# revision 41
# speedup vs baseline: 1.1532x; 1.0618x over previous
"""Trainium2 Bass kernel for grouped-query attention with qk-norm.

Problem (hardcoded): x(2,2048,1024) @ Wq(1024,1024) / Wkv(1024,512),
16 query heads, 4 kv heads, head_dim 64, k_scale(16,1,64) applied to the
group-broadcast k. Output (2,2048,1024).

Sharding: 8 cores = batch(2) x kv_heads(4). Each core computes its batch's
4 query heads against its kv head over the full 2048x2048 score matrix.

Device kernel layout choices:
- Host passes x transposed and kt-tiled (dim on partitions) so projection
  matmuls contract over dim with no on-device transposes, and all weights
  pre-tiled to [128, kt-major] so every input DMA is contiguous 1-8KB lines.
- k_scale is folded into Wq host-side ((q*ks)@k^T == q@(k*ks)^T), so k is
  projected once per kv head (64 wide) and shared by all 4 query heads;
  the k and v projections are column-tiled on the PE (both M=64) and run
  concurrently. kT is duplicated to partitions 64-127 by an SBUF-to-SBUF
  DMA so both QK row-tiles have a stationary copy.
- Scores are computed transposed (S^T: keys on partitions, queries free)
  so that exp(S^T) tiles feed the PV matmul directly as the moving
  operand (no P transpose). All matmul inputs are fp16 (1 row/cycle).
- Softmax skips the max-subtraction (inputs are bounded; exp stays well
  inside fp32 range) and normalizes after PV via an appended ones-row in
  the V stationary operand (row 64 of the PV psum accumulates sum(exp)).
- The first two attention blocks are interleaved with the projection
  chains so the ScalarE exp stream (the pipeline limiter) starts as soon
  as the first kv chunk is projected.
- Output is returned transposed per head (oT: 4*64 x 2048); the host
  transposes during the gather.
"""

from contextlib import ExitStack

import numpy as np

import concourse.bacc as bacc
import concourse.mybir as mybir
import concourse.tile as tile
from concourse.bass_utils import run_bass_kernel_spmd

# Problem constants
B, N, DIM = 2, 2048, 1024
HEADS, KV_HEADS, DH = 16, 4, 64
G = HEADS // KV_HEADS  # query heads per kv head (4)
NCORES = 8
P = 128
KT = DIM // P  # 8 contraction tiles over dim
IC = 512  # query-chunk width
NI = N // IC  # 4
NJ = N // P  # 16 key tiles
SCALE = DH**-0.5

F32 = mybir.dt.float32
F16 = mybir.dt.float16

DEBUG_DUMP = False


def emit_kernel(ctx, tc, xt, wq, wk, wv, eye, oT):
    nc = tc.nc
    Exp = mybir.ActivationFunctionType.Exp
    mult = mybir.AluOpType.mult

    wpool = ctx.enter_context(tc.tile_pool(name="w", bufs=1))
    qkpool = ctx.enter_context(tc.tile_pool(name="qk", bufs=1))
    ptpool = ctx.enter_context(tc.tile_pool(name="pt", bufs=6))
    npool = ctx.enter_context(tc.tile_pool(name="norm", bufs=2))

    # --- persistent SBUF tensors ---
    ones_sb = wpool.tile([P, DH], F16, tag="ones")
    eye_sb = wpool.tile([DH, DH], F16, tag="eye")
    wq_sb = wpool.tile([P, KT * 256], F16, tag="wq")
    wk_sb = wpool.tile([P, KT * 128], F16, tag="wk")  # [wk|wk] duplicated
    wv_sb = wpool.tile([P, KT * DH], F16, tag="wv")
    xts = wpool.tile([P, KT * N], F16, tag="xt")  # 4MB, [p, (ic, kt, c)]
    qT = [qkpool.tile([P, N], F16, name=f"qT{hp}", tag=f"qT{hp}") for hp in range(2)]
    kkT = qkpool.tile([P, N], F16, tag="kkT")  # kT on both partition halves
    vT_sb = qkpool.tile([DH, N], F16, tag="vT")
    vaug = qkpool.tile([P, NJ * (DH + 1)], F16, tag="vaug")
    nc.any.memset(vaug[:], 1.0)
    nc.any.memset(ones_sb[:], 1.0)
    warm = qkpool.tile([1, 2], F32, tag="warm")
    nc.any.memset(warm[0:1, 0:1], 0.0)
    nc.scalar.activation(warm[0:1, 1:2], warm[0:1, 0:1], Exp)

    # --- input DMAs: each queue runs ~150GB/s, so order each ring's FIFO
    # by when the data is needed: wk then x(ic0) race on both rings; wq/wv/
    # eye follow; later x chunks are emitted lazily in the ramp schedule ---
    XW = KT * IC  # columns per ic-chunk of xts

    def dma_x(ic):
        for h in range(2):
            eng = nc.sync if h == 0 else nc.gpsimd
            c0 = ic * XW + h * (XW // 2)
            eng.dma_start(xts[:, c0 : c0 + XW // 2], xt[:, c0 : c0 + XW // 2])

    WQW = KT * 256
    nc.sync.dma_start(wk_sb[:], wk[:, :])
    dma_x(0)
    nc.gpsimd.dma_start(wv_sb[:], wv[:, :])
    # wq split by head-pair so the hp0 chain can start sooner
    nc.gpsimd.dma_start(
        wq_sb[:].rearrange("p (k c) -> p k c", k=KT)[:, :, 0:128],
        wq[:, :].rearrange("p (k c) -> p k c", k=KT)[:, :, 0:128],
    )
    nc.sync.dma_start(eye_sb[:], eye[:, :])
    nc.gpsimd.dma_start(
        wq_sb[:].rearrange("p (k c) -> p k c", k=KT)[:, :, 128:256],
        wq[:, :].rearrange("p (k c) -> p k c", k=KT)[:, :, 128:256],
    )

    # --- psum pools (8 banks total: st 4 + proj 2 + PV accumulators 2) ---
    apsum = ctx.enter_context(tc.tile_pool(name="ap", bufs=2, space="PSUM"))
    pp = ctx.enter_context(tc.tile_pool(name="pp", bufs=2, space="PSUM"))
    opool = ctx.enter_context(tc.tile_pool(name="op", bufs=1, space="PSUM"))

    # Dummy matmuls during the initial DMA wait keep the PE HAM activity
    # monitor busy so real projections start at 2.4GHz instead of 1.2.
    for _ in range(14):
        wt = apsum.tile([DH, IC], F32, tag="s", name="wt", bufs=2)
        nc.tensor.matmul(
            wt[:, 0:DH], ones_sb[:, 0:DH], ones_sb[:, 0:DH],
            start=True, stop=True,
        )

    # --- projection chains ---
    def q_chain(hp, ic):
        csl = slice(ic * IC, (ic + 1) * IC)
        ps = pp.tile([P, IC], F32, tag="pj", name="pjq", bufs=2)
        for kt in range(KT):
            c0 = kt * 256 + hp * 128
            nc.tensor.matmul(
                ps[:],
                wq_sb[:, c0 : c0 + 128],
                xts[:, (ic * KT + kt) * IC : (ic * KT + kt + 1) * IC],
                start=(kt == 0),
                stop=(kt == KT - 1),
            )
        nc.vector.tensor_copy(qT[hp][:, csl], ps[:])

    # kt order: the second half of each x chunk lands on the gpsimd ring
    # (less queued ahead of it), so contract those first
    KT_ORDER = [4, 5, 6, 7, 0, 1, 2, 3]

    def k_mms(ic):
        # k projection with host-duplicated [wk|wk] stationary (M=128, same
        # cycles as M=64): psum rows 0-63 AND 64-127 both get kT, so one
        # copy fills both QK row-tile stationary halves.
        csl = slice(ic * IC, (ic + 1) * IC)
        ps_k = pp.tile([P, IC], F32, tag="pj", name="pjk", bufs=2)
        for i, kt in enumerate(KT_ORDER):
            xs = xts[:, (ic * KT + kt) * IC : (ic * KT + kt + 1) * IC]
            nc.tensor.matmul(
                ps_k[:, :],
                wk_sb[:, kt * 128 : (kt + 1) * 128],
                xs,
                start=(i == 0),
                stop=(i == KT - 1),
            )
        nc.vector.tensor_copy(kkT[:, csl], ps_k[:, :])

    def v_mms(ic):
        csl = slice(ic * IC, (ic + 1) * IC)
        ps_v = pp.tile([P, IC], F32, tag="pj", name="pjv", bufs=2)
        for i, kt in enumerate(KT_ORDER):
            xs = xts[:, (ic * KT + kt) * IC : (ic * KT + kt + 1) * IC]
            nc.tensor.matmul(
                ps_v[0:DH, :],
                wv_sb[:, kt * DH : (kt + 1) * DH],
                xs,
                start=(i == 0),
                stop=(i == KT - 1),
            )
        nc.vector.tensor_copy(vT_sb[:, csl], ps_v[0:DH, :])

    def kv_tail(ic):
        # vaug tiles (v transposed, with the ones-row kept from the memset)
        for jt in range(4 * ic, 4 * ic + 4):
            pv = pp.tile([P, DH], F16, tag="pj", bufs=2, name="pvt")
            nc.tensor.transpose(pv[:], vT_sb[:, jt * P : (jt + 1) * P], eye_sb[:])
            nc.vector.tensor_copy(
                vaug[:, jt * (DH + 1) : jt * (DH + 1) + DH], pv[:]
            )

    # --- attention ---
    def qk_exp(hp, ic, jt, pt):
        csl = slice(ic * IC, (ic + 1) * IC)
        st = apsum.tile([P, 2 * IC], F32, tag="s", bufs=2, name="st")
        for half in range(2):
            rsl = slice(half * 64, half * 64 + 64)
            nc.tensor.matmul(
                st[:, half * IC : (half + 1) * IC],
                kkT[rsl, jt * P : (jt + 1) * P],
                qT[hp][rsl, csl],
                start=True,
                stop=True,
                tile_position=(half * 64, 0),
            )
        nc.scalar.activation(pt[:], st[:], Exp, scale=SCALE)

    def pv_mm(o_ps, jt, pt):
        for half in range(2):
            nc.tensor.matmul(
                o_ps[half][:],
                vaug[:, jt * (DH + 1) : (jt + 1) * (DH + 1)],
                pt[:, half * IC : (half + 1) * IC],
                start=(jt == 0),
                stop=(jt == NJ - 1),
            )

    def attn4(hp, ic, o_ps, j0):
        pts = []
        for jt in range(j0, j0 + 4):
            pt = ptpool.tile([P, 2 * IC], F16, tag="pt")
            qk_exp(hp, ic, jt, pt)
            pts.append(pt)
        return pts

    def pv4(o_ps, j0, pts):
        for jt, pt in zip(range(j0, j0 + 4), pts):
            pv_mm(o_ps, jt, pt)

    def new_ops():
        return [
            opool.tile([DH + 1, IC], F32, name=f"ops{i}", tag=f"ops{i}", bufs=1)
            for i in range(2)
        ]

    def normalize_half(h, ic, fo):
        # GpSimd broadcasts the sums row across partitions (PE-free), then
        # a single-pass approx reciprocal (~18 bits) and the final multiply.
        # partition_broadcast needs its source on partition 0 (HW reads
        # channel 0 regardless of the AP base), so stage the row first.
        csl = slice(ic * IC, (ic + 1) * IC)
        srow = npool.tile([1, IC], F32, tag="srow", bufs=4)
        nc.vector.tensor_copy(srow[:], fo[DH : DH + 1, :])
        bc = npool.tile([DH, IC], F32, name="bcg", tag="bcg", bufs=4)
        nc.gpsimd.partition_broadcast(bc[:], srow[:])
        rb = npool.tile([DH, IC], F32, tag="rb", bufs=4)
        nc.vector.reciprocal_approx_fast(rb[:], bc[:])
        fin = npool.tile([DH, IC], F32, tag="fin", bufs=4)
        nc.vector.tensor_tensor(fin[:], fo[0:DH, :], rb[:], mult)
        nc.sync.dma_start(oT[h * DH : (h + 1) * DH, csl], fin[:])

    def drain_block(hp, ic, o_ps):
        # copy out of PSUM promptly so the next block's PV can start
        for half in range(2):
            fo = npool.tile([DH + 1, IC], F32, tag="fo", bufs=2, name="fo")
            nc.vector.tensor_copy(fo[:], o_ps[half][:])
            normalize_half(2 * hp + half, ic, fo)

    # --- ramp: first two attention blocks interleaved with projections.
    # Block (0,0) is PE-overcommitted (key production gates it); q chains
    # for later blocks are deferred into the PE-light blocks that follow.
    k_mms(0)
    q_chain(0, 0)
    dma_x(1)
    o_ps = new_ops()
    pts = attn4(0, 0, o_ps, 0)
    v_mms(0)
    kv_tail(0)
    pv4(o_ps, 0, pts)
    k_mms(1)
    pts = attn4(0, 0, o_ps, 4)
    dma_x(2)
    v_mms(1)
    kv_tail(1)
    pv4(o_ps, 4, pts)
    k_mms(2)
    pts = attn4(0, 0, o_ps, 8)
    dma_x(3)
    v_mms(2)
    kv_tail(2)
    pv4(o_ps, 8, pts)
    k_mms(3)
    pts = attn4(0, 0, o_ps, 12)
    v_mms(3)
    kv_tail(3)
    pv4(o_ps, 12, pts)
    q_chain(1, 0)
    drain_block(0, 0, o_ps)

    # block (1, 0) interleaved with the next q chains (near PE-balanced)
    o_ps = new_ops()
    pts = attn4(1, 0, o_ps, 0)
    q_chain(0, 1)
    pv4(o_ps, 0, pts)
    pts = attn4(1, 0, o_ps, 4)
    q_chain(1, 1)
    pv4(o_ps, 4, pts)
    pts = attn4(1, 0, o_ps, 8)
    q_chain(0, 2)
    pv4(o_ps, 8, pts)
    pts = attn4(1, 0, o_ps, 12)
    pv4(o_ps, 12, pts)
    drain_block(1, 0, o_ps)

    # --- remaining blocks (last q chains ride in the first two) ---
    DEFER = {(0, 1): [(1, 2)], (1, 1): [(0, 3), (1, 3)]}
    for ic in range(1, NI):
        for hp in range(2):
            o_ps = new_ops()
            for j0 in range(0, NJ, 4):
                pts = attn4(hp, ic, o_ps, j0)
                for qhp, qic in DEFER.pop((hp, ic), []) if j0 == 4 else []:
                    q_chain(qhp, qic)
                pv4(o_ps, j0, pts)
            drain_block(hp, ic, o_ps)

    if DEBUG_DUMP:
        for name, t, shape in [
            ("dbg_wq", wq_sb, (P, KT * 256)), ("dbg_wk", wk_sb, (P, KT * DH)),
            ("dbg_wv", wv_sb, (P, KT * DH)), ("dbg_kkT", kkT, (P, N)),
            ("dbg_qT0", qT[0], (P, N)), ("dbg_qT1", qT[1], (P, N)),
            ("dbg_vT", vT_sb, (DH, N)), ("dbg_vaug", vaug, (P, NJ * (DH + 1))),
            ("dbg_xts", xts, (P, KT * N)),
        ]:
            d = nc.dram_tensor(name, shape, F16, kind="ExternalOutput").ap()
            nc.sync.dma_start(d[:, :], t[:])


_CACHE = {}


def build():
    if "nc" in _CACHE:
        return _CACHE["nc"]
    nc = bacc.Bacc(
        "TRN2", target_bir_lowering=False, debug=False, num_devices=NCORES
    )
    xt = nc.dram_tensor("xt", (P, KT * N), F16, kind="ExternalInput").ap()
    wq = nc.dram_tensor("wq", (P, KT * 256), F16, kind="ExternalInput").ap()
    wk = nc.dram_tensor("wk", (P, KT * 128), F16, kind="ExternalInput").ap()
    wv = nc.dram_tensor("wv", (P, KT * DH), F16, kind="ExternalInput").ap()
    eye = nc.dram_tensor("eye", (DH, DH), F16, kind="ExternalInput").ap()
    oT = nc.dram_tensor("oT", (G * DH, N), F32, kind="ExternalOutput").ap()
    with tile.TileContext(nc) as tc:
        with ExitStack() as ctx:
            emit_kernel(ctx, tc, xt, wq, wk, wv, eye, oT)
    nc.compile()
    _CACHE["nc"] = nc
    return nc


def _tile_kt(w):
    # (1024, C) -> (128, KT*C): row-block kt lands at column block kt
    C = w.shape[1]
    return np.ascontiguousarray(
        w.reshape(KT, P, C).transpose(1, 0, 2).reshape(P, KT * C)
    )


def make_in_maps(x, Wq, Wkv, k_scale):
    x = np.asarray(x, dtype=np.float32)
    Wq = np.asarray(Wq, dtype=np.float32)
    Wkv = np.asarray(Wkv, dtype=np.float32)
    k_scale = np.asarray(k_scale, dtype=np.float32)
    # x[b].T tiled to [p, (ic, kt, c)] so each ic-chunk is one contiguous DMA
    xts = []
    for b in range(B):
        xT = x[b].T.reshape(KT, P, NI, IC)
        xts.append(
            np.ascontiguousarray(xT.transpose(1, 2, 0, 3).reshape(P, KT * N)).astype(
                np.float16
            )
        )
    in_maps = []
    for c in range(NCORES):
        b, kv = divmod(c, KV_HEADS)
        # fold the per-query-head k_scale into Wq: (q*ks)@k^T == q@(k*ks)^T
        wq_c = np.concatenate(
            [
                Wq[:, (kv * G + j) * DH : (kv * G + j + 1) * DH]
                * k_scale[kv * G + j, 0][None, :]
                for j in range(G)
            ],
            axis=1,
        )
        wk_c = Wkv[:, kv * DH : (kv + 1) * DH]
        wv_c = Wkv[:, KV_HEADS * DH + kv * DH : KV_HEADS * DH + (kv + 1) * DH]
        in_maps.append(
            {
                "xt": xts[b],
                "wq": _tile_kt(wq_c).astype(np.float16),
                "wk": _tile_kt(np.concatenate([wk_c, wk_c], axis=1)).astype(
                    np.float16
                ),
                "wv": _tile_kt(wv_c).astype(np.float16),
                "eye": np.eye(DH, dtype=np.float16),
            }
        )
    return in_maps


def gather(results):
    out = np.empty((B, N, HEADS * DH), dtype=np.float32)
    for c in range(NCORES):
        b, kv = divmod(c, KV_HEADS)
        out[b, :, kv * G * DH : (kv + 1) * G * DH] = results[c]["oT"].T
    return out


def kernel(x, Wq, Wkv, k_scale, _trace=False):
    nc = build()
    in_maps = make_in_maps(x, Wq, Wkv, k_scale)
    res = run_bass_kernel_spmd(
        nc, in_maps, core_ids=list(range(NCORES)), trace=_trace
    )
    out = gather(res.results)
    if _trace:
        kernel.last_result = res
    return out


# revision 45
# speedup vs baseline: 1.1625x; 1.0080x over previous
"""Trainium2 Bass kernel for grouped-query attention with qk-norm.

Problem (hardcoded): x(2,2048,1024) @ Wq(1024,1024) / Wkv(1024,512),
16 query heads, 4 kv heads, head_dim 64, k_scale(16,1,64) applied to the
group-broadcast k. Output (2,2048,1024).

Sharding: 8 cores = batch(2) x kv_heads(4). Each core computes its batch's
4 query heads against its kv head over the full 2048x2048 score matrix.

Device kernel layout choices:
- Host passes x transposed and kt-tiled (dim on partitions) so projection
  matmuls contract over dim with no on-device transposes, and all weights
  pre-tiled to [128, kt-major] so every input DMA is contiguous 1-8KB lines.
- k_scale is folded into Wq host-side ((q*ks)@k^T == q@(k*ks)^T), so k is
  projected once per kv head (64 wide) and shared by all 4 query heads;
  the k and v projections are column-tiled on the PE (both M=64) and run
  concurrently. kT is duplicated to partitions 64-127 by an SBUF-to-SBUF
  DMA so both QK row-tiles have a stationary copy.
- Scores are computed transposed (S^T: keys on partitions, queries free)
  so that exp(S^T) tiles feed the PV matmul directly as the moving
  operand (no P transpose). All matmul inputs are fp16 (1 row/cycle).
- Softmax skips the max-subtraction (inputs are bounded; exp stays well
  inside fp32 range) and normalizes after PV via an appended ones-row in
  the V stationary operand (row 64 of the PV psum accumulates sum(exp)).
- The first two attention blocks are interleaved with the projection
  chains so the ScalarE exp stream (the pipeline limiter) starts as soon
  as the first kv chunk is projected.
- Output is returned transposed per head (oT: 4*64 x 2048); the host
  transposes during the gather.
"""

from contextlib import ExitStack

import numpy as np

import concourse.bacc as bacc
import concourse.mybir as mybir
import concourse.tile as tile
from concourse.bass_utils import run_bass_kernel_spmd

# Problem constants
B, N, DIM = 2, 2048, 1024
HEADS, KV_HEADS, DH = 16, 4, 64
G = HEADS // KV_HEADS  # query heads per kv head (4)
NCORES = 8
P = 128
KT = DIM // P  # 8 contraction tiles over dim
IC = 512  # query-chunk width
NI = N // IC  # 4
NJ = N // P  # 16 key tiles
SCALE = DH**-0.5

F32 = mybir.dt.float32
F16 = mybir.dt.float16

DEBUG_DUMP = False


def emit_kernel(ctx, tc, xt, wq, wk, wv, eye, oT):
    nc = tc.nc
    Exp = mybir.ActivationFunctionType.Exp
    mult = mybir.AluOpType.mult

    wpool = ctx.enter_context(tc.tile_pool(name="w", bufs=1))
    qkpool = ctx.enter_context(tc.tile_pool(name="qk", bufs=1))
    ptpool = ctx.enter_context(tc.tile_pool(name="pt", bufs=6))
    npool = ctx.enter_context(tc.tile_pool(name="norm", bufs=2))

    # --- persistent SBUF tensors ---
    ones_sb = wpool.tile([P, DH], F16, tag="ones")
    eye_sb = wpool.tile([DH, DH], F16, tag="eye")
    wq_sb = wpool.tile([P, KT * 256], F16, tag="wq")
    wk_sb = wpool.tile([P, KT * 128], F16, tag="wk")  # [wk|wk] duplicated
    wv_sb = wpool.tile([P, KT * DH], F16, tag="wv")
    xts = wpool.tile([P, KT * N], F16, tag="xt")  # 4MB, [p, (ic, kt, c)]
    qT = [qkpool.tile([P, N], F16, name=f"qT{hp}", tag=f"qT{hp}") for hp in range(2)]
    kkT = qkpool.tile([P, N], F16, tag="kkT")  # kT on both partition halves
    vT_sb = qkpool.tile([DH, N], F16, tag="vT")
    vaug = qkpool.tile([P, NJ * (DH + 1)], F16, tag="vaug")
    nc.any.memset(vaug[:], 1.0)
    nc.any.memset(ones_sb[:], 1.0)
    warm = qkpool.tile([1, 2], F32, tag="warm")
    nc.any.memset(warm[0:1, 0:1], 0.0)
    nc.scalar.activation(warm[0:1, 1:2], warm[0:1, 0:1], Exp)

    # --- input DMAs: each queue runs ~150GB/s, so order each ring's FIFO
    # by when the data is needed: wk then x(ic0) race on both rings; wq/wv/
    # eye follow; later x chunks are emitted lazily in the ramp schedule ---
    XW = KT * IC  # columns per ic-chunk of xts

    def dma_x(ic):
        for h in range(2):
            eng = nc.sync if h == 0 else nc.gpsimd
            c0 = ic * XW + h * (XW // 2)
            eng.dma_start(xts[:, c0 : c0 + XW // 2], xt[:, c0 : c0 + XW // 2])

    nc.sync.dma_start(wk_sb[:], wk[:, :])
    dma_x(0)
    # wq hp0 half rides the otherwise-idle Scalar hwdge queue (done before
    # the exp stream starts); hp1 follows x(ic0) on gpsimd
    nc.scalar.dma_start(
        wq_sb[:].rearrange("p (k c) -> p k c", k=KT)[:, :, 0:128],
        wq[:, :].rearrange("p (k c) -> p k c", k=KT)[:, :, 0:128],
    )
    nc.gpsimd.dma_start(wv_sb[:], wv[:, :])
    nc.sync.dma_start(eye_sb[:], eye[:, :])
    nc.gpsimd.dma_start(
        wq_sb[:].rearrange("p (k c) -> p k c", k=KT)[:, :, 128:256],
        wq[:, :].rearrange("p (k c) -> p k c", k=KT)[:, :, 128:256],
    )

    # --- psum pools (8 banks total: st 4 + proj 2 + PV accumulators 2) ---
    apsum = ctx.enter_context(tc.tile_pool(name="ap", bufs=2, space="PSUM"))
    pp = ctx.enter_context(tc.tile_pool(name="pp", bufs=2, space="PSUM"))
    opool = ctx.enter_context(tc.tile_pool(name="op", bufs=1, space="PSUM"))

    # Dummy matmuls during the initial DMA wait keep the PE HAM activity
    # monitor busy (near-100% duty via N=512 moving) so the projection
    # chains start at 2.4GHz instead of 1.2.
    warm_mv = wpool.tile([P, IC], F16, tag="warmmv")
    nc.any.memset(warm_mv[:], 0.0)
    for _ in range(10):
        wt = apsum.tile([DH, IC], F32, tag="s", name="wt", bufs=2)
        nc.tensor.matmul(
            wt[:], ones_sb[:, 0:DH], warm_mv[:],
            start=True, stop=True,
        )

    # --- projection chains ---
    def q_chain(hp, ic):
        csl = slice(ic * IC, (ic + 1) * IC)
        ps = pp.tile([P, IC], F32, tag="pj", name="pjq", bufs=2)
        for kt in range(KT):
            c0 = kt * 256 + hp * 128
            nc.tensor.matmul(
                ps[:],
                wq_sb[:, c0 : c0 + 128],
                xts[:, (ic * KT + kt) * IC : (ic * KT + kt + 1) * IC],
                start=(kt == 0),
                stop=(kt == KT - 1),
            )
        nc.vector.tensor_copy(qT[hp][:, csl], ps[:])

    # kt order: the second half of each x chunk lands on the gpsimd ring
    # (less queued ahead of it), so contract those first
    KT_ORDER = [4, 5, 6, 7, 0, 1, 2, 3]

    def k_mms(ic):
        # k projection with host-duplicated [wk|wk] stationary (M=128, same
        # cycles as M=64): psum rows 0-63 AND 64-127 both get kT, so one
        # copy fills both QK row-tile stationary halves.
        csl = slice(ic * IC, (ic + 1) * IC)
        ps_k = pp.tile([P, IC], F32, tag="pj", name="pjk", bufs=2)
        for i, kt in enumerate(KT_ORDER):
            xs = xts[:, (ic * KT + kt) * IC : (ic * KT + kt + 1) * IC]
            nc.tensor.matmul(
                ps_k[:, :],
                wk_sb[:, kt * 128 : (kt + 1) * 128],
                xs,
                start=(i == 0),
                stop=(i == KT - 1),
            )
        nc.vector.tensor_copy(kkT[:, csl], ps_k[:, :])

    def v_mms(ic):
        csl = slice(ic * IC, (ic + 1) * IC)
        ps_v = pp.tile([P, IC], F32, tag="pj", name="pjv", bufs=2)
        for i, kt in enumerate(KT_ORDER):
            xs = xts[:, (ic * KT + kt) * IC : (ic * KT + kt + 1) * IC]
            nc.tensor.matmul(
                ps_v[0:DH, :],
                wv_sb[:, kt * DH : (kt + 1) * DH],
                xs,
                start=(i == 0),
                stop=(i == KT - 1),
            )
        nc.vector.tensor_copy(vT_sb[:, csl], ps_v[0:DH, :])

    def kv_tail(ic):
        # vaug tiles (v transposed, with the ones-row kept from the memset)
        for jt in range(4 * ic, 4 * ic + 4):
            pv = pp.tile([P, DH], F16, tag="pj", bufs=2, name="pvt")
            nc.tensor.transpose(pv[:], vT_sb[:, jt * P : (jt + 1) * P], eye_sb[:])
            nc.vector.tensor_copy(
                vaug[:, jt * (DH + 1) : jt * (DH + 1) + DH], pv[:]
            )

    # --- attention ---
    def qk_exp(hp, ic, jt, pt):
        csl = slice(ic * IC, (ic + 1) * IC)
        st = apsum.tile([P, 2 * IC], F32, tag="s", bufs=2, name="st")
        for half in range(2):
            rsl = slice(half * 64, half * 64 + 64)
            nc.tensor.matmul(
                st[:, half * IC : (half + 1) * IC],
                kkT[rsl, jt * P : (jt + 1) * P],
                qT[hp][rsl, csl],
                start=True,
                stop=True,
                tile_position=(half * 64, 0),
            )
        nc.scalar.activation(pt[:], st[:], Exp, scale=SCALE)

    def pv_mm(o_ps, jt, pt):
        for half in range(2):
            nc.tensor.matmul(
                o_ps[half][:],
                vaug[:, jt * (DH + 1) : (jt + 1) * (DH + 1)],
                pt[:, half * IC : (half + 1) * IC],
                start=(jt == 0),
                stop=(jt == NJ - 1),
            )

    def attn4(hp, ic, o_ps, j0):
        pts = []
        for jt in range(j0, j0 + 4):
            pt = ptpool.tile([P, 2 * IC], F16, tag="pt")
            qk_exp(hp, ic, jt, pt)
            pts.append(pt)
        return pts

    def pv4(o_ps, j0, pts):
        for jt, pt in zip(range(j0, j0 + 4), pts):
            pv_mm(o_ps, jt, pt)

    def new_ops():
        return [
            opool.tile([DH + 1, IC], F32, name=f"ops{i}", tag=f"ops{i}", bufs=1)
            for i in range(2)
        ]

    def normalize_half(h, ic, fo):
        # GpSimd broadcasts the sums row across partitions (PE-free), then
        # a single-pass approx reciprocal (~18 bits) and the final multiply.
        # partition_broadcast needs its source on partition 0 (HW reads
        # channel 0 regardless of the AP base), so stage the row first.
        csl = slice(ic * IC, (ic + 1) * IC)
        srow = npool.tile([1, IC], F32, tag="srow", bufs=4)
        nc.vector.tensor_copy(srow[:], fo[DH : DH + 1, :])
        bc = npool.tile([DH, IC], F32, name="bcg", tag="bcg", bufs=4)
        nc.gpsimd.partition_broadcast(bc[:], srow[:])
        rb = npool.tile([DH, IC], F32, tag="rb", bufs=4)
        nc.vector.reciprocal_approx_fast(rb[:], bc[:])
        fin = npool.tile([DH, IC], F32, tag="fin", bufs=4)
        nc.vector.tensor_tensor(fin[:], fo[0:DH, :], rb[:], mult)
        nc.sync.dma_start(oT[h * DH : (h + 1) * DH, csl], fin[:])

    def drain_block(hp, ic, o_ps, last=False):
        # copy out of PSUM promptly so the next block's PV can start; the
        # final block skips the copy (nothing waits on its banks) and
        # normalizes straight from PSUM to shorten the tail
        for half in range(2):
            if last:
                normalize_half(2 * hp + half, ic, o_ps[half])
            else:
                fo = npool.tile([DH + 1, IC], F32, tag="fo", bufs=2, name="fo")
                nc.vector.tensor_copy(fo[:], o_ps[half][:])
                normalize_half(2 * hp + half, ic, fo)

    # --- ramp: first two attention blocks interleaved with projections.
    # Block (0,0) is PE-overcommitted (key production gates it); q chains
    # for later blocks are deferred into the PE-light blocks that follow.
    k_mms(0)
    q_chain(0, 0)
    dma_x(1)
    o_ps = new_ops()
    pts = attn4(0, 0, o_ps, 0)
    v_mms(0)
    kv_tail(0)
    pv4(o_ps, 0, pts)
    k_mms(1)
    pts = attn4(0, 0, o_ps, 4)
    dma_x(2)
    v_mms(1)
    kv_tail(1)
    pv4(o_ps, 4, pts)
    k_mms(2)
    pts = attn4(0, 0, o_ps, 8)
    dma_x(3)
    v_mms(2)
    kv_tail(2)
    pv4(o_ps, 8, pts)
    k_mms(3)
    pts = attn4(0, 0, o_ps, 12)
    v_mms(3)
    kv_tail(3)
    pv4(o_ps, 12, pts)
    q_chain(1, 0)
    drain_block(0, 0, o_ps)

    # block (1, 0) interleaved with the next q chains (near PE-balanced)
    o_ps = new_ops()
    pts = attn4(1, 0, o_ps, 0)
    q_chain(0, 1)
    pv4(o_ps, 0, pts)
    pts = attn4(1, 0, o_ps, 4)
    q_chain(1, 1)
    pv4(o_ps, 4, pts)
    pts = attn4(1, 0, o_ps, 8)
    q_chain(0, 2)
    pv4(o_ps, 8, pts)
    pts = attn4(1, 0, o_ps, 12)
    pv4(o_ps, 12, pts)
    drain_block(1, 0, o_ps)

    # --- remaining blocks (last q chains ride in the first two) ---
    DEFER = {(0, 1): [(1, 2)], (1, 1): [(0, 3), (1, 3)]}
    for ic in range(1, NI):
        for hp in range(2):
            o_ps = new_ops()
            for j0 in range(0, NJ, 4):
                pts = attn4(hp, ic, o_ps, j0)
                for qhp, qic in DEFER.pop((hp, ic), []) if j0 == 4 else []:
                    q_chain(qhp, qic)
                pv4(o_ps, j0, pts)
            drain_block(hp, ic, o_ps, last=(ic == NI - 1 and hp == 1))

    if DEBUG_DUMP:
        for name, t, shape in [
            ("dbg_wq", wq_sb, (P, KT * 256)), ("dbg_wk", wk_sb, (P, KT * DH)),
            ("dbg_wv", wv_sb, (P, KT * DH)), ("dbg_kkT", kkT, (P, N)),
            ("dbg_qT0", qT[0], (P, N)), ("dbg_qT1", qT[1], (P, N)),
            ("dbg_vT", vT_sb, (DH, N)), ("dbg_vaug", vaug, (P, NJ * (DH + 1))),
            ("dbg_xts", xts, (P, KT * N)),
        ]:
            d = nc.dram_tensor(name, shape, F16, kind="ExternalOutput").ap()
            nc.sync.dma_start(d[:, :], t[:])


_CACHE = {}


def build():
    if "nc" in _CACHE:
        return _CACHE["nc"]
    nc = bacc.Bacc(
        "TRN2", target_bir_lowering=False, debug=False, num_devices=NCORES
    )
    xt = nc.dram_tensor("xt", (P, KT * N), F16, kind="ExternalInput").ap()
    wq = nc.dram_tensor("wq", (P, KT * 256), F16, kind="ExternalInput").ap()
    wk = nc.dram_tensor("wk", (P, KT * 128), F16, kind="ExternalInput").ap()
    wv = nc.dram_tensor("wv", (P, KT * DH), F16, kind="ExternalInput").ap()
    eye = nc.dram_tensor("eye", (DH, DH), F16, kind="ExternalInput").ap()
    oT = nc.dram_tensor("oT", (G * DH, N), F32, kind="ExternalOutput").ap()
    with tile.TileContext(nc) as tc:
        with ExitStack() as ctx:
            emit_kernel(ctx, tc, xt, wq, wk, wv, eye, oT)
    nc.compile()
    _CACHE["nc"] = nc
    return nc


def _tile_kt(w):
    # (1024, C) -> (128, KT*C): row-block kt lands at column block kt
    C = w.shape[1]
    return np.ascontiguousarray(
        w.reshape(KT, P, C).transpose(1, 0, 2).reshape(P, KT * C)
    )


def make_in_maps(x, Wq, Wkv, k_scale):
    x = np.asarray(x, dtype=np.float32)
    Wq = np.asarray(Wq, dtype=np.float32)
    Wkv = np.asarray(Wkv, dtype=np.float32)
    k_scale = np.asarray(k_scale, dtype=np.float32)
    # x[b].T tiled to [p, (ic, kt, c)] so each ic-chunk is one contiguous DMA
    xts = []
    for b in range(B):
        xT = x[b].T.reshape(KT, P, NI, IC)
        xts.append(
            np.ascontiguousarray(xT.transpose(1, 2, 0, 3).reshape(P, KT * N)).astype(
                np.float16
            )
        )
    in_maps = []
    for c in range(NCORES):
        b, kv = divmod(c, KV_HEADS)
        # fold the per-query-head k_scale into Wq: (q*ks)@k^T == q@(k*ks)^T
        wq_c = np.concatenate(
            [
                Wq[:, (kv * G + j) * DH : (kv * G + j + 1) * DH]
                * k_scale[kv * G + j, 0][None, :]
                for j in range(G)
            ],
            axis=1,
        )
        wk_c = Wkv[:, kv * DH : (kv + 1) * DH]
        wv_c = Wkv[:, KV_HEADS * DH + kv * DH : KV_HEADS * DH + (kv + 1) * DH]
        in_maps.append(
            {
                "xt": xts[b],
                "wq": _tile_kt(wq_c).astype(np.float16),
                "wk": _tile_kt(np.concatenate([wk_c, wk_c], axis=1)).astype(
                    np.float16
                ),
                "wv": _tile_kt(wv_c).astype(np.float16),
                "eye": np.eye(DH, dtype=np.float16),
            }
        )
    return in_maps


def gather(results):
    out = np.empty((B, N, HEADS * DH), dtype=np.float32)
    for c in range(NCORES):
        b, kv = divmod(c, KV_HEADS)
        out[b, :, kv * G * DH : (kv + 1) * G * DH] = results[c]["oT"].T
    return out


def kernel(x, Wq, Wkv, k_scale, _trace=False):
    nc = build()
    in_maps = make_in_maps(x, Wq, Wkv, k_scale)
    res = run_bass_kernel_spmd(
        nc, in_maps, core_ids=list(range(NCORES)), trace=_trace
    )
    out = gather(res.results)
    if _trace:
        kernel.last_result = res
    return out
